# revision 9
# baseline (speedup 1.0000x reference)
"""Causal multi-head attention (B=2, S=2048, D=1024, H=16) on 8 TRN2 NeuronCores.

Sharding: batch*heads across cores. Core c handles batch c//4 and the 4 heads
g*4..g*4+3 where g = c%4. Weights are sliced per core (Megatron-style column
split of Wq/Wk/Wv, row split of Wo); each core produces a partial projected
output [D, S] (transposed) and the host sums the 4 partials per batch.

Everything on-chip is kept transposed ([feature, seq]) so no transposes are
ever needed on the forward path:
  qT/kT = wq/wk^T @ xT            (PE, fp16, contraction over D)
  v     = x @ Wv^T                (fp16; s on partitions, + ones col appended)
  sT    = k @ qT  [s_k=128, s_q]  (PE, contraction over dh=64, 2 heads packed
                                   via base-partition 0/64 row groups)
  eT    = exp(sT/8) -> fp16, then an on-chip 0/1 mask (built once with
          affine_select) zeroes the 128-wide causal band (GPSIMD)
  avB   = eT-subtile^T @ v_aug    [q=128, 65]  per (head, q-sub): q on
          PARTITIONS, so the matmul free size is 65, not 512 -- half the PE
          cycles of the [65, 512] orientation, and the softmax denominator
          (col 64) comes along free.
  o     = avB[:, 0:64] * recip(avB[:, 64]) broadcast along the FREE dim (DVE)
          -- no denominator-broadcast matmul needed at all.
  oT    = PE-transpose(o, identity)  [dims 128, q 128] per q-sub (128 cyc)
  partialT = wo^T-chunks @ oT     (PE, fp16, contraction over 256 head dims)

All 16-bit tensors are fp16 (not bf16): same PE/DMA cost, 4 more mantissa
bits. Partials stream out as fp16 (host sums in float64).

Scheduling: engines execute their instruction streams in emission order, so
the kernel is emitted as ONE interleaved stream. Attention chunks (latency-
bound on the PE->ACT->GPSIMD->PE chain) are diluted with fine-grained filler
units (2-4 matmuls each): QKV work for later tiles, deferred Wo work for
earlier tiles, and deferred per-phase normalization units (fins: fin_a =
den/recip/divide on ACT+DVE, fin_b = PE transposes + oT copy, popped 1 and 3
chunks into the next phase respectively so the PE never parks on the DVE
chain). A per-phase emission-time ledger of ACT-vs-PE nanoseconds pumps
fillers exactly when the scalar engine (exp) would fall behind; AV matmuls
trail their scores by AV_LAG chunks. The chunk-major prologue advances all
four k/q accumulators per arriving x-block pair. The final phase staggers
normalization per q-sub (q-sub s's AV accumulation closes at diag chunk
12+s), interleaving per-sub fins with the AV drain, then runs Wo u=0 across 8
borrowed PSUM slots, per-sub u=1 closes, and 4 per-sub output DMAs split
across the SP (HWDGE) and Pool (SWDGE) queues.
"""

from collections import deque

import numpy as np

import concourse.bass as bass
import concourse.mybir as mybir
import concourse.tile as tile
from concourse import bacc, masks
from concourse.bass_utils import run_bass_kernel_spmd

B = 2
S = 2048
D = 1024
H = 16
DH = 64
N_CORES = 8
HG = H // 4  # 4 heads per core
GM = 4 * DH  # 256 head dims per core
FP32 = mybir.dt.float32
F16 = mybir.dt.float16

S_TILE = 512  # q-tile width (PSUM bank)
N_ST = S // S_TILE  # 4
KC = 128  # k-chunk (partition dim of scoresT)
N_KC = S // KC  # 16
N_DC = D // 128  # 8 d-chunks
AV_LAG = 4  # chunks between scores and their AV matmuls (hides exp+mask latency)


def build_program():
    nc = bacc.Bacc("TRN2", target_bir_lowering=False, debug=False)

    xT = nc.dram_tensor("xT", [D, S], F16, kind="ExternalInput")
    wq = nc.dram_tensor("wq", [D, GM], F16, kind="ExternalInput")
    wk = nc.dram_tensor("wk", [D, GM], F16, kind="ExternalInput")
    wv = nc.dram_tensor("wv", [D, GM], F16, kind="ExternalInput")
    wo = nc.dram_tensor("wo", [GM, D], F16, kind="ExternalInput")
    outT = nc.dram_tensor("outT", [D, S], F16, kind="ExternalOutput")

    with tile.TileContext(nc) as tc:
        with (
            tc.tile_pool(name="persist", bufs=1) as persist,
            tc.tile_pool(name="xb", bufs=8) as xb_pool,
            tc.tile_pool(name="exp", bufs=10) as exp_pool,
            tc.tile_pool(name="small", bufs=4) as small_pool,
            tc.tile_pool(name="outsb", bufs=4) as out_pool,
            tc.tile_pool(name="mm", bufs=2, space="PSUM") as mm_pool,
            tc.tile_pool(name="scores", bufs=2, space="PSUM") as sc_pool,
            tc.tile_pool(name="av", bufs=2, space="PSUM") as av_pool,
        ):
            # ---- persistent SBUF tensors ----
            wo_sb = persist.tile([128, 2, D], F16, tag="wo")
            ones_col = persist.tile([128, 1], FP32, tag="ones")
            ident = persist.tile([128, 128], F16, tag="ident")
            nc.vector.memset(ones_col[:, :], 1.0)
            masks.make_identity(nc, ident[:, :])
            # causal mask patterns, generated on-chip (no DMA):
            # mask4[p, j, q] = 1.0 iff 128*j + p <= q
            mask4 = persist.tile([128, 4, S_TILE], FP32, tag="mask4")
            nc.gpsimd.memset(mask4[:, :, :], 1.0)
            for j in range(4):
                nc.gpsimd.affine_select(
                    mask4[:, j, :],
                    mask4[:, j, :],
                    pattern=[[1, S_TILE]],
                    compare_op=mybir.AluOpType.is_ge,
                    fill=0.0,
                    base=-128 * j,
                    channel_multiplier=-1,
                )
            w_sb = {}
            for name in ("q", "k", "v"):
                w_sb[name] = persist.tile(
                    [128, N_DC, GM], F16, tag=f"w{name}", name=f"w{name}sb"
                )

            qT = {}  # (u, t) -> [128, 512]   2 heads stacked (rows 0-63 / 64-127)
            kT = {}
            vt = {}  # c16 -> [128, HG, 65]   v chunk with ones col per head
            oT = {}  # (u, t) -> [128, 512]   attention out, dims on partitions
            for t in range(N_ST):
                for u in range(2):
                    qT[(u, t)] = persist.tile(
                        [128, S_TILE], F16, tag=f"qT{u}{t}", name=f"qT{u}{t}"
                    )
                    kT[(u, t)] = persist.tile(
                        [128, S_TILE], F16, tag=f"kT{u}{t}", name=f"kT{u}{t}"
                    )
                    oT[(u, t)] = persist.tile(
                        [128, S_TILE], F16, tag=f"oT{u}{t}", name=f"oT{u}{t}"
                    )
            for c16 in range(N_KC):
                vt[c16] = persist.tile(
                    [128, HG, DH + 1], F16, tag=f"v{c16}", name=f"v{c16}"
                )

            xb = {}

            def load_xb(t, c0):
                # two d-chunks per DMA: halves the HWDGE issue cost (625ns
                # per DMA vs 364ns fp16 transfer, so issue rate dominates)
                blk = xb_pool.tile(
                    [128, 2, S_TILE], F16, tag="xb", name=f"xb{c0}_{t}"
                )
                nc.sync.dma_start(
                    blk[:, :, :],
                    xT[
                        c0 * 128 : (c0 + 2) * 128,
                        t * S_TILE : (t + 1) * S_TILE,
                    ].rearrange("(i p) m -> p i m", i=2),
                    )
                xb[(c0, t)] = blk[:, 0, :]
                xb[(c0 + 1, t)] = blk[:, 1, :]

            # ---- DMA stream order ----
            # tile-0 inputs first (weights in halves interleaved with x blocks
            # so the first k/q matmuls start ~3.5us in), then x tiles 1..3
            # stream ahead of their QKV filler units, then wo (needed late).
            wk_r = wk.rearrange("(c p) m -> p c m", p=128)
            wq_r = wq.rearrange("(c p) m -> p c m", p=128)
            wv_r = wv.rearrange("(c p) m -> p c m", p=128)
            # paced for the chunk-major prologue: per 2-chunk period, the k and
            # q weight chunks land just before the x block pair that uses them
            for p in range(4):
                nc.sync.dma_start(
                    w_sb["k"][:, 2 * p : 2 * p + 2, :], wk_r[:, 2 * p : 2 * p + 2, :]
                )
                if p > 0:
                    nc.sync.dma_start(
                        w_sb["q"][:, 2 * p : 2 * p + 2, :],
                        wq_r[:, 2 * p : 2 * p + 2, :],
                    )
                load_xb(0, 2 * p)
                if p == 0:
                    nc.sync.dma_start(
                        w_sb["q"][:, 0:2, :], wq_r[:, 0:2, :]
                    )
            nc.sync.dma_start(w_sb["v"][:, 0:4, :], wv_r[:, 0:4, :])
            nc.sync.dma_start(w_sb["v"][:, 4:8, :], wv_r[:, 4:8, :])
            for c0 in range(0, N_DC, 2):
                load_xb(1, c0)
            nc.sync.dma_start(wo_sb[:, :, :], wo.rearrange("(u p) d -> p u d", p=128))
            for t in range(2, N_ST):
                for c0 in range(0, N_DC, 2):
                    load_xb(t, c0)

            # ---- emission thunks ----
            # Group PSUM tiles are created lazily by the first unit of each
            # group (cell dict) so mm_pool's buffer rotation follows actual
            # use order, not enqueue order.
            def qk_matmuls(name, u, t, cell, c0, c1):
                if c0 == 0:
                    cell["ps"] = mm_pool.tile(
                        [128, S_TILE], FP32, tag="mm", name=f"ps{name}{u}{t}"
                    )
                ps = cell["ps"]
                for c in range(c0, c1):
                    nc.tensor.matmul(
                        ps[:, :],
                        lhsT=w_sb[name][:, c, u * 128 : (u + 1) * 128],
                        rhs=xb[(c, t)][:, :],
                        start=(c == 0),
                        stop=(c == N_DC - 1),
                    )
                if c1 == N_DC:
                    dst = kT if name == "k" else qT
                    nc.vector.tensor_copy(dst[(u, t)][:, :], ps[:, :])

            def v_matmuls(t, s4, cell, c0, c1):
                c16 = 4 * t + s4
                if c0 == 0:
                    cell["ps"] = mm_pool.tile(
                        [128, GM], FP32, tag="mm", name=f"psv{c16}"
                    )
                ps = cell["ps"]
                for c in range(c0, c1):
                    nc.tensor.matmul(
                        ps[:, :],
                        lhsT=xb[(c, t)][:, s4 * 128 : (s4 + 1) * 128],
                        rhs=w_sb["v"][:, c, :],
                        start=(c == 0),
                        stop=(c == N_DC - 1),
                    )
                if c1 == N_DC:
                    nc.vector.tensor_copy(
                        vt[c16][:, :, 0:DH], ps.rearrange("p (h d) -> p h d", h=HG)
                    )
                    nc.gpsimd.tensor_copy(
                        vt[c16][:, :, DH : DH + 1],
                        ones_col[:, 0:1].broadcast_to((128, HG, 1)),
                    )

            def emit_wo_group(t, dc, on_act=False):
                po = mm_pool.tile([128, S_TILE], FP32, tag="mm", name=f"po{t}{dc}")
                for u in range(2):
                    nc.tensor.matmul(
                        po[:, :],
                        lhsT=wo_sb[:, u, dc * 128 : (dc + 1) * 128],
                        rhs=oT[(u, t)][:, :],
                        start=(u == 0),
                        stop=(u == 1),
                    )
                ob = out_pool.tile([128, S_TILE], F16, tag="ob")
                if on_act:  # tail: split the copy across ACT+DVE so the PSUM
                    # bank frees at PE pace, not copy pace
                    nc.scalar.copy(ob[:, 0 : S_TILE // 2], po[:, 0 : S_TILE // 2])
                    nc.vector.tensor_copy(ob[:, S_TILE // 2 :], po[:, S_TILE // 2 :])
                else:
                    nc.vector.tensor_copy(ob[:, :], po[:, :])
                nc.sync.dma_start(
                    outT[dc * 128 : (dc + 1) * 128, t * S_TILE : (t + 1) * S_TILE],
                    ob[:, :],
                )

            # filler queue: (tile_tag, cost_cycles, thunk). Attention chunks pump
            # filler units between chunks to keep the PE dense while exp/mask
            # latency elapses. QKV units are tagged with their tile (flushed
            # before that tile's attention); Wo units are tagged 99 (pump/drain
            # only -- they are enqueued once their oT inputs exist).
            fillers = deque()

            def enqueue_qkv_units(t):
                for name in ("k", "q"):
                    for u in range(2):
                        cell = {}
                        for c0 in range(0, N_DC, 2):
                            fillers.append(
                                (
                                    t,
                                    2 * S_TILE,
                                    lambda name=name, u=u, t=t, cell=cell, c0=c0: qk_matmuls(
                                        name, u, t, cell, c0, c0 + 2
                                    ),
                                )
                            )
                for s4 in range(4):
                    cell = {}
                    for c0 in range(0, N_DC, 4):
                        fillers.append(
                            (
                                t,
                                4 * GM,
                                lambda t=t, s4=s4, cell=cell, c0=c0: v_matmuls(
                                    t, s4, cell, c0, c0 + 4
                                ),
                            )
                        )

            # --- emission-time ACT-vs-PE ledger (reset per attention phase).
            # exp work accumulates act_ns; attention matmuls + pumped fillers
            # accumulate pe_ns. Pump fillers whenever ACT is ahead, so the PE
            # stream is diluted exactly where the scalar engine needs time.
            PE_CY = 1.0 / 2.4  # ns per cycle at peak
            SLACK = 900.0  # exp-pipeline fill depth: don't pump before ACT is
            # genuinely ahead of the PE stream in wall-clock terms
            ledger = {"pe": 0.0, "act": -SLACK}

            # deferred normalization units: (min_chunk, cost, thunk) popped by
            # pumps min_chunk chunks into the NEXT phase, when their upstream
            # ACT/DVE chain has drained
            fins = deque()
            phase_chunk = {"c": 0}

            def pump_ledger(max_tag, extra=0.0):
                ledger["act"] += extra
                while ledger["pe"] < ledger["act"] and (fins or fillers):
                    if fins and phase_chunk["c"] >= fins[0][0]:
                        _, cost, thunk = fins.popleft()
                    elif fillers:
                        tag, cost, thunk = fillers[0]
                        if tag == 99 and fins:
                            # a wo(t) unit must never be emitted while a fin
                            # producing its oT input is still queued: the PE
                            # stream is in-order, so that would deadlock
                            return
                        if tag != 99 and tag > max_tag:
                            return
                        fillers.popleft()
                    else:
                        return
                    thunk()
                    ledger["pe"] += cost * PE_CY

            def flush_tile(t):
                # 99 = wo units (pump/drain only, reserved as late filler)
                while fillers and fillers[0][0] <= t:
                    _, _, thunk = fillers.popleft()
                    thunk()

            # prologue, chunk-major: all four k/q accumulation groups advance
            # per arriving x block (their PSUM lives in sc_pool, idle during
            # the prologue), so the PE has ~850ns of work per ~1.5us DMA chunk
            # instead of ~430ns. v groups become tile-0 fillers.
            sc0 = sc_pool.tile([128, 2 * S_TILE], FP32, tag="sc", name="pro_sc0")
            sc1 = sc_pool.tile([128, 2 * S_TILE], FP32, tag="sc", name="pro_sc1")
            pro = {("k", 0): sc0[:, 0:S_TILE], ("q", 0): sc0[:, S_TILE:],
                   ("k", 1): sc1[:, 0:S_TILE], ("q", 1): sc1[:, S_TILE:]}
            for s4 in range(4):
                cell = {}
                for c0 in range(0, N_DC, 4):
                    fillers.append(
                        (
                            0,
                            4 * GM,
                            lambda s4=s4, cell=cell, c0=c0: v_matmuls(
                                0, s4, cell, c0, c0 + 4
                            ),
                        )
                    )
            for c in range(N_DC):
                for name in ("k", "q"):
                    for u in range(2):
                        nc.tensor.matmul(
                            pro[(name, u)],
                            lhsT=w_sb[name][:, c, u * 128 : (u + 1) * 128],
                            rhs=xb[(c, 0)][:, :],
                            start=(c == 0),
                            stop=(c == N_DC - 1),
                        )
            for u in range(2):  # ACT is idle during the prologue; keep DVE free
                nc.scalar.copy(kT[(u, 0)][:, :], pro[("k", u)])
                nc.scalar.copy(qT[(u, 0)][:, :], pro[("q", u)])
            for t in range(1, N_ST):
                enqueue_qkv_units(t)

            for t in range(N_ST):
                nch = 4 * t + 4
                for hp in range(2):
                    u = hp
                    final = t == N_ST - 1 and hp == 1
                    if t > 0 or hp > 0:
                        flush_tile(t)  # qkv(<=t) must be emitted
                    ledger["pe"] = 0.0
                    ledger["act"] = -SLACK
                    phase_chunk["c"] = 0
                    avs = [
                        av_pool.tile(
                            [128, 4, DH + 1], FP32, tag="av", name=f"av{t}{hp}{i}"
                        )
                        for i in range(2)
                    ]
                    pending_avs = deque()  # AV trails scores by AV_LAG chunks

                    def emit_av(cc, exx, jj, avs=avs, hp=hp, t=t, nch=nch):
                        # q-on-partitions AV: per (head, q-sub 128) the matmul
                        # free size is just 65 (64 dims + the ones/denominator
                        # col); q-sub s's accumulation closes at diag chunk
                        # 4t+s. PSUM accumulation-group starts are ZERO-REGION
                        # (2KB bank) granular: exactly ONE start and ONE stop
                        # per av tile -- the bank-wide pending-zero from the
                        # single start zeroes each sub-region on first touch.
                        s0 = max(jj, 0)
                        for i in range(2):
                            for s in range(s0, 4):
                                nc.tensor.matmul(
                                    avs[i][:, s, :],
                                    lhsT=exx[
                                        :,
                                        i * S_TILE + 128 * s : i * S_TILE
                                        + 128 * (s + 1),
                                    ],
                                    rhs=vt[cc][:, 2 * hp + i, :],
                                    start=(cc == 0 and s == s0),
                                    stop=(cc == nch - 1 and s == 3),
                                    skip_group_check=True,
                                )
                        ledger["pe"] += 2 * (4 - s0) * (DH + 1) * PE_CY

                    for c in range(nch):
                        # Diagonal chunks only touch q columns >= 128j
                        # (causal): scores / exp / AV skip the masked prefix.
                        j = c - 4 * t
                        q0 = 128 * j if j >= 0 else 0
                        w = S_TILE - q0
                        sc = sc_pool.tile([128, 2 * S_TILE], FP32, tag="sc")
                        for i in range(2):  # head parity: rows 0-63 / 64-127
                            bp = 64 * i
                            nc.tensor.matmul(
                                sc[:, i * S_TILE + q0 : (i + 1) * S_TILE],
                                lhsT=kT[(u, c // 4)][
                                    bp : bp + DH, (c % 4) * 128 : (c % 4 + 1) * 128
                                ],
                                rhs=qT[(u, t)][bp : bp + DH, q0:],
                                start=True,
                                stop=True,
                            )
                        ledger["pe"] += 2 * w * PE_CY
                        ex = exp_pool.tile([128, 2 * S_TILE], F16, tag="ex")
                        exv = ex.rearrange("p (i n) -> p i n", i=2)[:, :, q0:]
                        scv = sc.rearrange("p (i n) -> p i n", i=2)[:, :, q0:]
                        nc.scalar.activation(
                            exv, scv, mybir.ActivationFunctionType.Exp, scale=0.125
                        )
                        ledger["act"] += 2 * w * 0.833 + 242
                        if j >= 0:
                            # zero the causal triangle (mask is 0/1, exact).
                            # Only the band [q0, 128j+128) needs masking; one
                            # op covers both head slots via a broadcast mask.
                            bhi = 128 * j + 128
                            exb = ex.rearrange("p (i n) -> p i n", i=2)[
                                :, :, q0:bhi
                            ]
                            nc.gpsimd.tensor_mul(
                                exb,
                                exb,
                                mask4[:, j : j + 1, q0:bhi].broadcast_to(
                                    (128, 2, bhi - q0)
                                ),
                            )

                        pending_avs.append((c, ex, j))
                        phase_chunk["c"] = c
                        pump_ledger(t + 1)
                        if len(pending_avs) > AV_LAG:
                            emit_av(*pending_avs.popleft())
                    if t == 0 and hp == 0:
                        flush_tile(0)  # v(t0) needed by the AV drain below

                    if not final:
                        while pending_avs:
                            emit_av(*pending_avs.popleft())
                            # the exp tail is still draining on ACT in
                            # wall-clock terms; keep filler between the drain
                            pump_ledger(t + 1, extra=700)
                        # normalization: rec = 1/den per (head, q-sub) on DVE,
                        # o = av * rec via FREE-dim broadcast (q is on
                        # partitions), then 4 PE transposes into oT. Split so
                        # the PE piece (fin_b) pops 3 chunks into the next
                        # phase, when fin_a's DVE chain has drained.
                        cell = {}

                        def emit_fin_a(avs=avs, cell=cell):
                            den = small_pool.tile([128, 2, 4], FP32, tag="den")
                            for i in range(2):
                                nc.scalar.copy(den[:, i, :], avs[i][:, :, DH])
                            rec = small_pool.tile([128, 2, 4], FP32, tag="rec32")
                            nc.vector.reciprocal_approx_fast(
                                rec[:, :, :], den[:, :, :]
                            )
                            ob2 = small_pool.tile([128, 4, 128], F16, tag="obo")
                            for i in range(2):
                                nc.vector.tensor_mul(
                                    ob2[:, :, 64 * i : 64 * i + DH],
                                    avs[i][:, :, 0:DH],
                                    rec[:, i, :]
                                    .rearrange("p (a b) -> p a b", b=1)
                                    .broadcast_to((128, 4, DH)),
                                )
                            cell["ob2"] = ob2

                        def emit_fin_b(cell=cell, u=u, t=t):
                            # one mm_pool tile per transpose: each PE
                            # transpose is its own accumulation group and a
                            # group start claims a whole 2KB zero region, so
                            # outputs must not share a PSUM bank
                            ob2 = cell["ob2"]
                            for s in range(4):
                                pt = mm_pool.tile(
                                    [128, 128], F16, tag="mm", name=f"pt{t}{u}{s}"
                                )
                                nc.tensor.matmul(
                                    pt[:, :],
                                    lhsT=ob2[:, s, :],
                                    rhs=ident[:, :],
                                    is_transpose=True,
                                )
                                nc.vector.tensor_copy(
                                    oT[(u, t)][:, 128 * s : 128 * (s + 1)],
                                    pt[:, :],
                                )

                        fins.append((1, 0, emit_fin_a))
                        fins.append((3, 4 * 128, emit_fin_b))
                    else:
                        # ---- final phase: staggered per-q-sub normalization
                        # interleaved with the AV drain, then the Wo tail.
                        # Flush any queued fins (they produce oT(.,t3) that
                        # the tail's u0 matmuls read) and leftover wo units
                        # (their mm_pool allocations must all precede fpt's,
                        # or bufs=2 rotation would evict fpt mid-use).
                        while fins:
                            _, cost, thunk = fins.popleft()
                            thunk()
                            ledger["pe"] += cost * PE_CY
                        while fillers:
                            _, cost, thunk = fillers.popleft()
                            thunk()
                            ledger["pe"] += cost * PE_CY
                        fin_cells = {}

                        def fin_sub_a(s, avs=avs):
                            den = small_pool.tile(
                                [128, 2, 1], FP32, tag="den", name=f"fden{s}"
                            )
                            for i in range(2):
                                nc.scalar.copy(
                                    den[:, i, :], avs[i][:, s, DH : DH + 1]
                                )
                            rec = small_pool.tile(
                                [128, 2, 1], FP32, tag="rec32", name=f"frec{s}"
                            )
                            nc.vector.reciprocal_approx_fast(
                                rec[:, :, :], den[:, :, :]
                            )
                            ob2 = small_pool.tile(
                                [128, 1, 128], F16, tag="obo", name=f"fob{s}"
                            )
                            for i in range(2):
                                nc.vector.tensor_mul(
                                    ob2[:, 0, 64 * i : 64 * i + DH],
                                    avs[i][:, s, 0:DH],
                                    rec[:, i, :].broadcast_to((128, DH)),
                                )
                            fin_cells[s] = ob2

                        def fin_sub_b(s, u=u, t=t):
                            ob2 = fin_cells[s]
                            pt = mm_pool.tile(
                                [128, 128], F16, tag="mm", name=f"fpt{s}"
                            )
                            nc.tensor.matmul(
                                pt[:, :],
                                lhsT=ob2[:, 0, :],
                                rhs=ident[:, :],
                                is_transpose=True,
                            )
                            nc.vector.tensor_copy(
                                oT[(u, t)][:, 128 * s : 128 * (s + 1)],
                                pt[:, :],
                            )

                        while pending_avs:
                            cc, exx, jj = pending_avs.popleft()
                            emit_av(cc, exx, jj)
                            s_stop = cc - 4 * t
                            if s_stop >= 0:
                                fin_sub_a(s_stop)
                                if s_stop >= 1:
                                    fin_sub_b(s_stop - 1)
                            pump_ledger(t + 1, extra=700)

                        # kernel tail: last tile's 8 Wo groups across 8
                        # borrowed PSUM slots. u=0 (full width, needs only
                        # oT(0,t3)) runs while the staggered fin chains drain;
                        # u=1 closes per q-sub as its oT slice lands; outputs
                        # leave as 4 per-sub DMAs split across SP and Pool
                        # queues so no issue serialization at the very end.
                        tl = N_ST - 1
                        tail_a = sc_pool.tile(
                            [128, 2 * S_TILE], FP32, tag="sc", name="tail_a"
                        )
                        tail_b = sc_pool.tile(
                            [128, 2 * S_TILE], FP32, tag="sc", name="tail_b"
                        )
                        slots = [
                            tail_a[:, 0:S_TILE],
                            tail_a[:, S_TILE:],
                            tail_b[:, 0:S_TILE],
                            tail_b[:, S_TILE:],
                            None,  # mm slots allocated after fin_sub_b(3)
                            None,
                            None,  # av slots borrowed once avs are dead
                            None,
                        ]

                        def wo_u0(dc):
                            nc.tensor.matmul(
                                slots[dc],
                                lhsT=wo_sb[:, 0, dc * 128 : (dc + 1) * 128],
                                rhs=oT[(0, tl)][:, :],
                                start=True,
                                stop=False,
                                skip_group_check=True,
                            )

                        for dc in range(4):
                            wo_u0(dc)
                        fin_sub_b(3)  # before mm tail allocs (rotation safety)
                        slots[4] = mm_pool.tile(
                            [128, S_TILE], FP32, tag="mm", name="tail_m0"
                        )[:, :]
                        slots[5] = mm_pool.tile(
                            [128, S_TILE], FP32, tag="mm", name="tail_m1"
                        )[:, :]
                        slots[6] = av_pool.tile(
                            [128, S_TILE], FP32, tag="av", name="tail_v0"
                        )[:, :]
                        slots[7] = av_pool.tile(
                            [128, S_TILE], FP32, tag="av", name="tail_v1"
                        )[:, :]
                        for dc in range(4, 8):
                            wo_u0(dc)
                        for s in range(4):
                            for dc in range(N_DC):
                                nc.tensor.matmul(
                                    slots[dc][:, 128 * s : 128 * (s + 1)],
                                    lhsT=wo_sb[:, 1, dc * 128 : (dc + 1) * 128],
                                    rhs=oT[(1, tl)][:, 128 * s : 128 * (s + 1)],
                                    start=False,
                                    stop=(s == 3),
                                    skip_group_check=True,
                                )
                            ob_s = out_pool.tile(
                                [128, N_DC, 128], F16, tag="ob", name=f"obs{s}"
                            )
                            for dc in range(N_DC):
                                # alternate ACT/DVE so copies run concurrently
                                src = slots[dc][:, 128 * s : 128 * (s + 1)]
                                if dc % 2 == 0:
                                    nc.scalar.copy(ob_s[:, dc, :], src)
                                else:
                                    nc.vector.tensor_copy(ob_s[:, dc, :], src)
                            dst = outT[
                                :, tl * S_TILE + 128 * s : tl * S_TILE + 128 * (s + 1)
                            ].rearrange("(i p) m -> p i m", i=N_DC)
                            if s == 3:
                                nc.sync.dma_start(dst, ob_s[:, :, :])
                            else:
                                nc.gpsimd.dma_start(dst, ob_s[:, :, :])
                # Wo for this tile becomes filler work for later attention
                # (the last tile's Wo is the kernel tail, emitted above)
                if t < N_ST - 1:
                    for dc in range(N_DC):
                        fillers.append(
                            (99, 2 * S_TILE, lambda t=t, dc=dc: emit_wo_group(t, dc))
                        )
            while fillers:
                fillers.popleft()[2]()
            while fins:
                fins.popleft()[2]()
    nc.compile()
    return nc


_NC_CACHE = None


def _get_program():
    global _NC_CACHE
    if _NC_CACHE is None:
        _NC_CACHE = build_program()
    return _NC_CACHE


def _make_in_maps(x, Wq, Wk, Wv, Wo):
    f16 = np.float16
    xTs = [np.ascontiguousarray(x[b].T).astype(f16) for b in range(B)]
    in_maps = []
    for core in range(N_CORES):
        b, g = divmod(core, HG)
        r0, r1 = g * GM, (g + 1) * GM
        in_maps.append(
            {
                "xT": xTs[b],
                "wq": np.ascontiguousarray(Wq[r0:r1, :].T).astype(f16),
                "wk": np.ascontiguousarray(Wk[r0:r1, :].T).astype(f16),
                "wv": np.ascontiguousarray(Wv[r0:r1, :].T).astype(f16),
                "wo": np.ascontiguousarray(Wo[:, r0:r1].T).astype(f16),
            }
        )
    return in_maps


def kernel(x, Wq, Wk, Wv, Wo, **_unused):
    x = np.asarray(x, dtype=np.float32)
    Wq = np.asarray(Wq, dtype=np.float32)
    Wk = np.asarray(Wk, dtype=np.float32)
    Wv = np.asarray(Wv, dtype=np.float32)
    Wo = np.asarray(Wo, dtype=np.float32)

    nc = _get_program()
    in_maps = _make_in_maps(x, Wq, Wk, Wv, Wo)
    res = run_bass_kernel_spmd(nc, in_maps, core_ids=list(range(N_CORES)))
    out = np.zeros((B, S, D), dtype=np.float64)
    for core in range(N_CORES):
        b = core // HG
        out[b] += res.results[core]["outT"].T.astype(np.float64)
    return out.astype(np.float32)


# revision 14
# speedup vs baseline: 1.0511x; 1.0511x over previous
"""Causal multi-head attention (B=2, S=2048, D=1024, H=16) on 8 TRN2 NeuronCores.

Sharding: batch*heads across cores. Core c handles batch c//4 and the 4 heads
g*4..g*4+3 where g = c%4. Weights are sliced per core (Megatron-style column
split of Wq/Wk/Wv, row split of Wo); each core produces a partial projected
output [D, S] (transposed) and the host sums the 4 partials per batch.

Everything on-chip is kept transposed ([feature, seq]) so no transposes are
ever needed on the forward path:
  qT/kT = wq/wk^T @ xT            (PE, fp16, contraction over D)
  v     = x @ Wv^T                (fp16; s on partitions, + ones col appended)
  sT    = k @ qT  [s_k=128, s_q]  (PE, contraction over dh=64, 2 heads packed
                                   via base-partition 0/64 row groups)
  eT    = exp(sT/8) -> fp16, then an on-chip 0/1 mask (built once with
          affine_select) zeroes the 128-wide causal band (GPSIMD)
  avB   = eT-subtile^T @ v_aug    [q=128, 65]  per (head, q-sub): q on
          PARTITIONS, so the matmul free size is 65, not 512 -- half the PE
          cycles of the [65, 512] orientation, and the softmax denominator
          (col 64) comes along free.
  o     = avB[:, 0:64] * recip(avB[:, 64]) broadcast along the FREE dim (DVE)
          -- no denominator-broadcast matmul needed at all.
  oT    = PE-transpose(o, identity)  [dims 128, q 128] per q-sub (128 cyc)
  partialT = wo^T-chunks @ oT     (PE, fp16, contraction over 256 head dims)

All 16-bit tensors are fp16 (not bf16): same PE/DMA cost, 4 more mantissa
bits. Partials stream out as fp16 (host sums in float64).

Scheduling: engines execute their instruction streams in emission order, so
the kernel is emitted as ONE interleaved stream. Attention chunks (latency-
bound on the PE->ACT->GPSIMD->PE chain) are diluted with fine-grained filler
units (2-4 matmuls each): QKV work for later tiles, deferred Wo work for
earlier tiles, and deferred per-phase normalization units (fins: fin_a =
den/recip/divide on ACT+DVE, fin_b = PE transposes + oT copy, popped 1 and 3
chunks into the next phase respectively so the PE never parks on the DVE
chain). A per-phase emission-time ledger of ACT-vs-PE nanoseconds pumps
fillers exactly when the scalar engine (exp) would fall behind; AV matmuls
trail their scores by AV_LAG chunks. The chunk-major prologue advances all
four k/q accumulators per arriving x-block pair. The final phase staggers
normalization per q-sub (q-sub s's AV accumulation closes at diag chunk
12+s), interleaving per-sub fins with the AV drain, then runs Wo u=0 across 8
borrowed PSUM slots, per-sub u=1 closes, and 4 per-sub output DMAs split
across the SP (HWDGE) and Pool (SWDGE) queues.
"""

from collections import deque

import numpy as np

import concourse.bass as bass
import concourse.mybir as mybir
import concourse.tile as tile
from concourse import bacc, masks
from concourse.bass_utils import run_bass_kernel_spmd

B = 2
S = 2048
D = 1024
H = 16
DH = 64
N_CORES = 8
HG = H // 4  # 4 heads per core
GM = 4 * DH  # 256 head dims per core
FP32 = mybir.dt.float32
F16 = mybir.dt.float16

S_TILE = 512  # q-tile width (PSUM bank)
N_ST = S // S_TILE  # 4
KC = 128  # k-chunk (partition dim of scoresT)
N_KC = S // KC  # 16
N_DC = D // 128  # 8 d-chunks
AV_LAG = 4  # chunks between scores and their AV matmuls (hides exp+mask latency)


def build_program():
    nc = bacc.Bacc("TRN2", target_bir_lowering=False, debug=False)

    xT = nc.dram_tensor("xT", [D, S], F16, kind="ExternalInput")
    wq = nc.dram_tensor("wq", [D, GM], F16, kind="ExternalInput")
    wk = nc.dram_tensor("wk", [D, GM], F16, kind="ExternalInput")
    wv = nc.dram_tensor("wv", [D, GM], F16, kind="ExternalInput")
    wo = nc.dram_tensor("wo", [GM, D], F16, kind="ExternalInput")
    outT = nc.dram_tensor("outT", [D, S], F16, kind="ExternalOutput")

    with tile.TileContext(nc) as tc:
        with (
            tc.tile_pool(name="persist", bufs=1) as persist,
            tc.tile_pool(name="xb", bufs=8) as xb_pool,
            tc.tile_pool(name="exp", bufs=10) as exp_pool,
            tc.tile_pool(name="small", bufs=4) as small_pool,
            tc.tile_pool(name="outsb", bufs=4) as out_pool,
            tc.tile_pool(name="mm", bufs=2, space="PSUM") as mm_pool,
            tc.tile_pool(name="scores", bufs=2, space="PSUM") as sc_pool,
            tc.tile_pool(name="av", bufs=2, space="PSUM") as av_pool,
        ):
            # ---- persistent SBUF tensors ----
            wo_sb = persist.tile([128, 2, D], F16, tag="wo")
            ones_col = persist.tile([128, 1], FP32, tag="ones")
            ident = persist.tile([128, 128], F16, tag="ident")
            nc.vector.memset(ones_col[:, :], 1.0)
            masks.make_identity(nc, ident[:, :])
            # causal mask patterns, generated on-chip (no DMA):
            # mask4[p, j, q] = 1.0 iff 128*j + p <= q
            mask4 = persist.tile([128, 4, S_TILE], FP32, tag="mask4")
            nc.gpsimd.memset(mask4[:, :, :], 1.0)
            for j in range(4):
                nc.gpsimd.affine_select(
                    mask4[:, j, :],
                    mask4[:, j, :],
                    pattern=[[1, S_TILE]],
                    compare_op=mybir.AluOpType.is_ge,
                    fill=0.0,
                    base=-128 * j,
                    channel_multiplier=-1,
                )
            w_sb = {}
            for name in ("q", "k", "v"):
                w_sb[name] = persist.tile(
                    [128, N_DC, GM], F16, tag=f"w{name}", name=f"w{name}sb"
                )

            qT = {}  # (u, t) -> [128, 512]   2 heads stacked (rows 0-63 / 64-127)
            kT = {}
            vt = {}  # c16 -> [128, HG, 65]   v chunk with ones col per head
            oT = {}  # (u, t) -> [128, 512]   attention out, dims on partitions
            for t in range(N_ST):
                for u in range(2):
                    qT[(u, t)] = persist.tile(
                        [128, S_TILE], F16, tag=f"qT{u}{t}", name=f"qT{u}{t}"
                    )
                    kT[(u, t)] = persist.tile(
                        [128, S_TILE], F16, tag=f"kT{u}{t}", name=f"kT{u}{t}"
                    )
                    oT[(u, t)] = persist.tile(
                        [128, S_TILE], F16, tag=f"oT{u}{t}", name=f"oT{u}{t}"
                    )
            for c16 in range(N_KC):
                vt[c16] = persist.tile(
                    [128, HG, DH + 1], F16, tag=f"v{c16}", name=f"v{c16}"
                )

            xb = {}

            def load_xb(t, c0):
                # two d-chunks per DMA: halves the HWDGE issue cost (625ns
                # per DMA vs 364ns fp16 transfer, so issue rate dominates)
                blk = xb_pool.tile(
                    [128, 2, S_TILE], F16, tag="xb", name=f"xb{c0}_{t}"
                )
                nc.sync.dma_start(
                    blk[:, :, :],
                    xT[
                        c0 * 128 : (c0 + 2) * 128,
                        t * S_TILE : (t + 1) * S_TILE,
                    ].rearrange("(i p) m -> p i m", i=2),
                    )
                xb[(c0, t)] = blk[:, 0, :]
                xb[(c0 + 1, t)] = blk[:, 1, :]

            # ---- DMA stream order ----
            # tile-0 inputs first (weights in halves interleaved with x blocks
            # so the first k/q matmuls start ~3.5us in), then x tiles 1..3
            # stream ahead of their QKV filler units, then wo (needed late).
            wk_r = wk.rearrange("(c p) m -> p c m", p=128)
            wq_r = wq.rearrange("(c p) m -> p c m", p=128)
            wv_r = wv.rearrange("(c p) m -> p c m", p=128)
            # paced for the chunk-major prologue: per 2-chunk period, the k and
            # q weight chunks land just before the x block pair that uses them
            for p in range(4):
                nc.sync.dma_start(
                    w_sb["k"][:, 2 * p : 2 * p + 2, :], wk_r[:, 2 * p : 2 * p + 2, :]
                )
                if p > 0:
                    nc.sync.dma_start(
                        w_sb["q"][:, 2 * p : 2 * p + 2, :],
                        wq_r[:, 2 * p : 2 * p + 2, :],
                    )
                load_xb(0, 2 * p)
                if p == 0:
                    nc.sync.dma_start(
                        w_sb["q"][:, 0:2, :], wq_r[:, 0:2, :]
                    )
            nc.sync.dma_start(w_sb["v"][:, 0:4, :], wv_r[:, 0:4, :])
            nc.sync.dma_start(w_sb["v"][:, 4:8, :], wv_r[:, 4:8, :])
            for c0 in range(0, N_DC, 2):
                load_xb(1, c0)
            nc.sync.dma_start(wo_sb[:, :, :], wo.rearrange("(u p) d -> p u d", p=128))
            for t in range(2, N_ST):
                for c0 in range(0, N_DC, 2):
                    load_xb(t, c0)

            # ---- emission thunks ----
            # Group PSUM tiles are created lazily by the first unit of each
            # group (cell dict) so mm_pool's buffer rotation follows actual
            # use order, not enqueue order.
            def qk_matmuls(name, u, t, cell, c0, c1):
                if c0 == 0:
                    cell["ps"] = mm_pool.tile(
                        [128, S_TILE], FP32, tag="mm", name=f"ps{name}{u}{t}"
                    )
                ps = cell["ps"]
                for c in range(c0, c1):
                    nc.tensor.matmul(
                        ps[:, :],
                        lhsT=w_sb[name][:, c, u * 128 : (u + 1) * 128],
                        rhs=xb[(c, t)][:, :],
                        start=(c == 0),
                        stop=(c == N_DC - 1),
                    )
                if c1 == N_DC:
                    dst = kT if name == "k" else qT
                    nc.vector.tensor_copy(dst[(u, t)][:, :], ps[:, :])

            def v_matmuls(t, s4, cell, c0, c1):
                c16 = 4 * t + s4
                if c0 == 0:
                    cell["ps"] = mm_pool.tile(
                        [128, GM], FP32, tag="mm", name=f"psv{c16}"
                    )
                ps = cell["ps"]
                for c in range(c0, c1):
                    nc.tensor.matmul(
                        ps[:, :],
                        lhsT=xb[(c, t)][:, s4 * 128 : (s4 + 1) * 128],
                        rhs=w_sb["v"][:, c, :],
                        start=(c == 0),
                        stop=(c == N_DC - 1),
                    )
                if c1 == N_DC:
                    nc.vector.tensor_copy(
                        vt[c16][:, :, 0:DH], ps.rearrange("p (h d) -> p h d", h=HG)
                    )
                    nc.gpsimd.tensor_copy(
                        vt[c16][:, :, DH : DH + 1],
                        ones_col[:, 0:1].broadcast_to((128, HG, 1)),
                    )

            def emit_wo_group(t, dc, on_act=False):
                po = mm_pool.tile([128, S_TILE], FP32, tag="mm", name=f"po{t}{dc}")
                for u in range(2):
                    nc.tensor.matmul(
                        po[:, :],
                        lhsT=wo_sb[:, u, dc * 128 : (dc + 1) * 128],
                        rhs=oT[(u, t)][:, :],
                        start=(u == 0),
                        stop=(u == 1),
                    )
                ob = out_pool.tile([128, S_TILE], F16, tag="ob")
                if on_act:  # tail: split the copy across ACT+DVE so the PSUM
                    # bank frees at PE pace, not copy pace
                    nc.scalar.copy(ob[:, 0 : S_TILE // 2], po[:, 0 : S_TILE // 2])
                    nc.vector.tensor_copy(ob[:, S_TILE // 2 :], po[:, S_TILE // 2 :])
                else:
                    nc.vector.tensor_copy(ob[:, :], po[:, :])
                nc.sync.dma_start(
                    outT[dc * 128 : (dc + 1) * 128, t * S_TILE : (t + 1) * S_TILE],
                    ob[:, :],
                )

            # filler queue: (tile_tag, cost_cycles, thunk). Attention chunks pump
            # filler units between chunks to keep the PE dense while exp/mask
            # latency elapses. QKV units are tagged with their tile (flushed
            # before that tile's attention); Wo units are tagged 99 (pump/drain
            # only -- they are enqueued once their oT inputs exist).
            fillers = deque()

            def enqueue_qkv_units(t):
                for name in ("k", "q"):
                    for u in range(2):
                        cell = {}
                        for c0 in range(0, N_DC, 2):
                            fillers.append(
                                (
                                    t,
                                    2 * S_TILE,
                                    lambda name=name, u=u, t=t, cell=cell, c0=c0: qk_matmuls(
                                        name, u, t, cell, c0, c0 + 2
                                    ),
                                )
                            )
                # v(t) is only consumed by (t,0)'s AV drain, so its units are
                # tagged t+0.5: they skip the phase-start flush and instead
                # pump as filler DURING (t,0), where late phases are starved
                for s4 in range(4):
                    cell = {}
                    for c0 in range(0, N_DC, 4):
                        fillers.append(
                            (
                                t + 0.5,
                                4 * GM,
                                lambda t=t, s4=s4, cell=cell, c0=c0: v_matmuls(
                                    t, s4, cell, c0, c0 + 4
                                ),
                            )
                        )

            # --- emission-time ACT-vs-PE ledger (reset per attention phase).
            # exp work accumulates act_ns; attention matmuls + pumped fillers
            # accumulate pe_ns. Pump fillers whenever ACT is ahead, so the PE
            # stream is diluted exactly where the scalar engine needs time.
            PE_CY = 1.0 / 2.4  # ns per cycle at peak
            SLACK = 900.0  # exp-pipeline fill depth: don't pump before ACT is
            # genuinely ahead of the PE stream in wall-clock terms
            ledger = {"pe": 0.0, "act": -SLACK}

            # deferred normalization units: (min_chunk, cost, thunk) popped by
            # pumps min_chunk chunks into the NEXT phase, when their upstream
            # ACT/DVE chain has drained
            fins = deque()
            phase_chunk = {"c": 0}

            def pump_ledger(max_tag, extra=0.0):
                ledger["act"] += extra
                while ledger["pe"] < ledger["act"] and (fins or fillers):
                    if fins and phase_chunk["c"] >= fins[0][0]:
                        _, cost, thunk = fins.popleft()
                    elif fillers:
                        tag, cost, thunk = fillers[0]
                        if tag == 99 and fins:
                            # a wo(t) unit must never be emitted while a fin
                            # producing its oT input is still queued: the PE
                            # stream is in-order, so that would deadlock
                            return
                        if tag == 99 and max_tag < N_ST:
                            # hold wo units for the filler-starved t3 phases
                            return
                        if tag != 99 and tag > max_tag:
                            return
                        fillers.popleft()
                    else:
                        return
                    thunk()
                    ledger["pe"] += cost * PE_CY

            def flush_tile(t):
                # 99 = wo units (pump/drain only, reserved as late filler)
                while fillers and fillers[0][0] <= t:
                    _, _, thunk = fillers.popleft()
                    thunk()

            # prologue, chunk-major: all four k/q accumulation groups advance
            # per arriving x block (their PSUM lives in sc_pool, idle during
            # the prologue), so the PE has ~850ns of work per ~1.5us DMA chunk
            # instead of ~430ns. v groups become tile-0 fillers.
            sc0 = sc_pool.tile([128, 2 * S_TILE], FP32, tag="sc", name="pro_sc0")
            sc1 = sc_pool.tile([128, 2 * S_TILE], FP32, tag="sc", name="pro_sc1")
            pro = {("k", 0): sc0[:, 0:S_TILE], ("q", 0): sc0[:, S_TILE:],
                   ("k", 1): sc1[:, 0:S_TILE], ("q", 1): sc1[:, S_TILE:]}
            for s4 in range(4):
                cell = {}
                for c0 in range(0, N_DC, 4):
                    fillers.append(
                        (
                            0.5,
                            4 * GM,
                            lambda s4=s4, cell=cell, c0=c0: v_matmuls(
                                0, s4, cell, c0, c0 + 4
                            ),
                        )
                    )
            for c in range(N_DC):
                for name in ("k", "q"):
                    for u in range(2):
                        nc.tensor.matmul(
                            pro[(name, u)],
                            lhsT=w_sb[name][:, c, u * 128 : (u + 1) * 128],
                            rhs=xb[(c, 0)][:, :],
                            start=(c == 0),
                            stop=(c == N_DC - 1),
                        )
            for u in range(2):  # ACT is idle during the prologue; keep DVE free
                nc.scalar.copy(kT[(u, 0)][:, :], pro[("k", u)])
                nc.scalar.copy(qT[(u, 0)][:, :], pro[("q", u)])
            for t in range(1, N_ST):
                enqueue_qkv_units(t)

            for t in range(N_ST):
                nch = 4 * t + 4
                for hp in range(2):
                    u = hp
                    final = t == N_ST - 1 and hp == 1
                    if t > 0 or hp > 0:
                        flush_tile(t)  # qkv(<=t) must be emitted
                    ledger["pe"] = 0.0
                    ledger["act"] = -SLACK
                    phase_chunk["c"] = 0
                    avs = [
                        av_pool.tile(
                            [128, 4, DH + 1], FP32, tag="av", name=f"av{t}{hp}{i}"
                        )
                        for i in range(2)
                    ]
                    pending_avs = deque()  # AV trails scores by AV_LAG chunks

                    def emit_av(cc, exx, jj, avs=avs, hp=hp, t=t, nch=nch):
                        # q-on-partitions AV: per (head, q-sub 128) the matmul
                        # free size is just 65 (64 dims + the ones/denominator
                        # col); q-sub s's accumulation closes at diag chunk
                        # 4t+s. PSUM accumulation-group starts are ZERO-REGION
                        # (2KB bank) granular: exactly ONE start and ONE stop
                        # per av tile -- the bank-wide pending-zero from the
                        # single start zeroes each sub-region on first touch.
                        s0 = max(jj, 0)
                        for i in range(2):
                            for s in range(s0, 4):
                                nc.tensor.matmul(
                                    avs[i][:, s, :],
                                    lhsT=exx[
                                        :,
                                        i * S_TILE + 128 * s : i * S_TILE
                                        + 128 * (s + 1),
                                    ],
                                    rhs=vt[cc][:, 2 * hp + i, :],
                                    start=(cc == 0 and s == s0),
                                    stop=(cc == nch - 1 and s == 3),
                                    skip_group_check=True,
                                )
                        ledger["pe"] += 2 * (4 - s0) * (DH + 1) * PE_CY

                    for c in range(nch):
                        # Diagonal chunks only touch q columns >= 128j
                        # (causal): scores / exp / AV skip the masked prefix.
                        j = c - 4 * t
                        q0 = 128 * j if j >= 0 else 0
                        w = S_TILE - q0
                        sc = sc_pool.tile([128, 2 * S_TILE], FP32, tag="sc")
                        for i in range(2):  # head parity: rows 0-63 / 64-127
                            bp = 64 * i
                            nc.tensor.matmul(
                                sc[:, i * S_TILE + q0 : (i + 1) * S_TILE],
                                lhsT=kT[(u, c // 4)][
                                    bp : bp + DH, (c % 4) * 128 : (c % 4 + 1) * 128
                                ],
                                rhs=qT[(u, t)][bp : bp + DH, q0:],
                                start=True,
                                stop=True,
                            )
                        ledger["pe"] += 2 * w * PE_CY
                        ex = exp_pool.tile([128, 2 * S_TILE], F16, tag="ex")
                        exv = ex.rearrange("p (i n) -> p i n", i=2)[:, :, q0:]
                        scv = sc.rearrange("p (i n) -> p i n", i=2)[:, :, q0:]
                        nc.scalar.activation(
                            exv, scv, mybir.ActivationFunctionType.Exp, scale=0.125
                        )
                        ledger["act"] += 2 * w * 0.833 + 242
                        if j >= 0:
                            # zero the causal triangle (mask is 0/1, exact).
                            # Only the band [q0, 128j+128) needs masking; one
                            # op covers both head slots via a broadcast mask.
                            bhi = 128 * j + 128
                            exb = ex.rearrange("p (i n) -> p i n", i=2)[
                                :, :, q0:bhi
                            ]
                            nc.gpsimd.tensor_mul(
                                exb,
                                exb,
                                mask4[:, j : j + 1, q0:bhi].broadcast_to(
                                    (128, 2, bhi - q0)
                                ),
                            )

                        pending_avs.append((c, ex, j))
                        phase_chunk["c"] = c
                        pump_ledger(t + 1)
                        if len(pending_avs) > AV_LAG:
                            emit_av(*pending_avs.popleft())
                    if hp == 0:
                        flush_tile(t + 0.5)  # v(t) needed by the AV drain

                    if not final:
                        while pending_avs:
                            emit_av(*pending_avs.popleft())
                            # the exp tail is still draining on ACT in
                            # wall-clock terms; keep filler between the drain
                            pump_ledger(t + 1, extra=700)
                        # normalization: rec = 1/den per (head, q-sub) on DVE,
                        # o = av * rec via FREE-dim broadcast (q is on
                        # partitions), then 4 PE transposes into oT. Split so
                        # the PE piece (fin_b) pops 3 chunks into the next
                        # phase, when fin_a's DVE chain has drained.
                        cell = {}

                        def emit_fin_a(avs=avs, cell=cell):
                            den = small_pool.tile([128, 2, 4], FP32, tag="den")
                            for i in range(2):
                                nc.scalar.copy(den[:, i, :], avs[i][:, :, DH])
                            rec = small_pool.tile([128, 2, 4], FP32, tag="rec32")
                            nc.vector.reciprocal_approx_fast(
                                rec[:, :, :], den[:, :, :]
                            )
                            ob2 = small_pool.tile([128, 4, 128], F16, tag="obo")
                            for i in range(2):
                                nc.vector.tensor_mul(
                                    ob2[:, :, 64 * i : 64 * i + DH],
                                    avs[i][:, :, 0:DH],
                                    rec[:, i, :]
                                    .rearrange("p (a b) -> p a b", b=1)
                                    .broadcast_to((128, 4, DH)),
                                )
                            cell["ob2"] = ob2

                        def emit_fin_b(cell=cell, u=u, t=t):
                            # one mm_pool tile per transpose: each PE
                            # transpose is its own accumulation group and a
                            # group start claims a whole 2KB zero region, so
                            # outputs must not share a PSUM bank
                            ob2 = cell["ob2"]
                            for s in range(4):
                                pt = mm_pool.tile(
                                    [128, 128], F16, tag="mm", name=f"pt{t}{u}{s}"
                                )
                                nc.tensor.matmul(
                                    pt[:, :],
                                    lhsT=ob2[:, s, :],
                                    rhs=ident[:, :],
                                    is_transpose=True,
                                )
                                nc.vector.tensor_copy(
                                    oT[(u, t)][:, 128 * s : 128 * (s + 1)],
                                    pt[:, :],
                                )

                        fins.append((1, 0, emit_fin_a))
                        fins.append((3, 4 * 128, emit_fin_b))
                    else:
                        # ---- final phase: staggered per-q-sub normalization
                        # interleaved with the AV drain, then the Wo tail.
                        # Flush any queued fins (they produce oT(.,t3) that
                        # the tail's u0 matmuls read) and leftover wo units
                        # (their mm_pool allocations must all precede fpt's,
                        # or bufs=2 rotation would evict fpt mid-use).
                        while fins:
                            _, cost, thunk = fins.popleft()
                            thunk()
                            ledger["pe"] += cost * PE_CY
                        while fillers:
                            _, cost, thunk = fillers.popleft()
                            thunk()
                            ledger["pe"] += cost * PE_CY
                        fin_cells = {}

                        def fin_sub_a(s, avs=avs):
                            den = small_pool.tile(
                                [128, 2, 1], FP32, tag="den", name=f"fden{s}"
                            )
                            for i in range(2):
                                nc.scalar.copy(
                                    den[:, i, :], avs[i][:, s, DH : DH + 1]
                                )
                            rec = small_pool.tile(
                                [128, 2, 1], FP32, tag="rec32", name=f"frec{s}"
                            )
                            nc.vector.reciprocal_approx_fast(
                                rec[:, :, :], den[:, :, :]
                            )
                            ob2 = small_pool.tile(
                                [128, 1, 128], F16, tag="obo", name=f"fob{s}"
                            )
                            for i in range(2):
                                nc.vector.tensor_mul(
                                    ob2[:, 0, 64 * i : 64 * i + DH],
                                    avs[i][:, s, 0:DH],
                                    rec[:, i, :].broadcast_to((128, DH)),
                                )
                            fin_cells[s] = ob2

                        def fin_sub_b(s, u=u, t=t):
                            ob2 = fin_cells[s]
                            pt = mm_pool.tile(
                                [128, 128], F16, tag="mm", name=f"fpt{s}"
                            )
                            nc.tensor.matmul(
                                pt[:, :],
                                lhsT=ob2[:, 0, :],
                                rhs=ident[:, :],
                                is_transpose=True,
                            )
                            nc.vector.tensor_copy(
                                oT[(u, t)][:, 128 * s : 128 * (s + 1)],
                                pt[:, :],
                            )

                        while pending_avs:
                            cc, exx, jj = pending_avs.popleft()
                            emit_av(cc, exx, jj)
                            s_stop = cc - 4 * t
                            if s_stop >= 0:
                                fin_sub_a(s_stop)
                                if s_stop >= 1:
                                    fin_sub_b(s_stop - 1)
                            pump_ledger(t + 1, extra=700)

                        # kernel tail: last tile's 8 Wo groups across 8
                        # borrowed PSUM slots. u=0 (full width, needs only
                        # oT(0,t3)) runs while the staggered fin chains drain;
                        # u=1 closes per q-sub as its oT slice lands; outputs
                        # leave as 4 per-sub DMAs split across SP and Pool
                        # queues so no issue serialization at the very end.
                        tl = N_ST - 1
                        tail_a = sc_pool.tile(
                            [128, 2 * S_TILE], FP32, tag="sc", name="tail_a"
                        )
                        tail_b = sc_pool.tile(
                            [128, 2 * S_TILE], FP32, tag="sc", name="tail_b"
                        )
                        slots = [
                            tail_a[:, 0:S_TILE],
                            tail_a[:, S_TILE:],
                            tail_b[:, 0:S_TILE],
                            tail_b[:, S_TILE:],
                            None,  # mm slots allocated after fin_sub_b(3)
                            None,
                            None,  # av slots borrowed once avs are dead
                            None,
                        ]

                        def wo_u0(dc):
                            nc.tensor.matmul(
                                slots[dc],
                                lhsT=wo_sb[:, 0, dc * 128 : (dc + 1) * 128],
                                rhs=oT[(0, tl)][:, :],
                                start=True,
                                stop=False,
                                skip_group_check=True,
                            )

                        for dc in range(4):
                            wo_u0(dc)
                        fin_sub_b(3)  # before mm tail allocs (rotation safety)
                        slots[4] = mm_pool.tile(
                            [128, S_TILE], FP32, tag="mm", name="tail_m0"
                        )[:, :]
                        slots[5] = mm_pool.tile(
                            [128, S_TILE], FP32, tag="mm", name="tail_m1"
                        )[:, :]
                        slots[6] = av_pool.tile(
                            [128, S_TILE], FP32, tag="av", name="tail_v0"
                        )[:, :]
                        slots[7] = av_pool.tile(
                            [128, S_TILE], FP32, tag="av", name="tail_v1"
                        )[:, :]
                        for dc in range(4, 8):
                            wo_u0(dc)
                        # u=1 closes per q-sub; outputs leave as two half-
                        # tile DMAs [128, 8, 256] (512B descriptors -- no
                        # small-elem latency penalty) on separate queues
                        # (Pool SWDGE then SP HWDGE), each after its 2 subs.
                        obh = {}
                        for h in range(2):
                            obh[h] = out_pool.tile(
                                [128, N_DC, 256], F16, tag="ob", name=f"obh{h}"
                            )
                        for s in range(4):
                            for dc in range(N_DC):
                                nc.tensor.matmul(
                                    slots[dc][:, 128 * s : 128 * (s + 1)],
                                    lhsT=wo_sb[:, 1, dc * 128 : (dc + 1) * 128],
                                    rhs=oT[(1, tl)][:, 128 * s : 128 * (s + 1)],
                                    start=False,
                                    stop=(s == 3),
                                    skip_group_check=True,
                                )
                                if s % 2 == 1:
                                    # both subs of this half are closed for
                                    # this dc: copy [128, 256] out now so the
                                    # copies pipeline under later u1 matmuls
                                    src = slots[dc][:, 128 * (s - 1) : 128 * (s + 1)]
                                    dsth = obh[s // 2][:, dc, :]
                                    if dc % 2 == 0:
                                        nc.scalar.copy(dsth, src)
                                    else:
                                        nc.vector.tensor_copy(dsth, src)
                            if s % 2 == 1:
                                h = s // 2
                                dst = outT[
                                    :,
                                    tl * S_TILE + 256 * h : tl * S_TILE
                                    + 256 * (h + 1),
                                ].rearrange("(i p) m -> p i m", i=N_DC)
                                if h == 0:
                                    nc.gpsimd.dma_start(dst, obh[h][:, :, :])
                                else:
                                    nc.sync.dma_start(dst, obh[h][:, :, :])
                # Wo for this tile becomes filler work for later attention
                # (the last tile's Wo is the kernel tail, emitted above)
                if t < N_ST - 1:
                    for dc in range(N_DC):
                        fillers.append(
                            (99, 2 * S_TILE, lambda t=t, dc=dc: emit_wo_group(t, dc))
                        )
            while fillers:
                fillers.popleft()[2]()
            while fins:
                fins.popleft()[2]()
    nc.compile()
    return nc


_NC_CACHE = None


def _get_program():
    global _NC_CACHE
    if _NC_CACHE is None:
        _NC_CACHE = build_program()
    return _NC_CACHE


def _make_in_maps(x, Wq, Wk, Wv, Wo):
    f16 = np.float16
    xTs = [np.ascontiguousarray(x[b].T).astype(f16) for b in range(B)]
    in_maps = []
    for core in range(N_CORES):
        b, g = divmod(core, HG)
        r0, r1 = g * GM, (g + 1) * GM
        in_maps.append(
            {
                "xT": xTs[b],
                "wq": np.ascontiguousarray(Wq[r0:r1, :].T).astype(f16),
                "wk": np.ascontiguousarray(Wk[r0:r1, :].T).astype(f16),
                "wv": np.ascontiguousarray(Wv[r0:r1, :].T).astype(f16),
                "wo": np.ascontiguousarray(Wo[:, r0:r1].T).astype(f16),
            }
        )
    return in_maps


def kernel(x, Wq, Wk, Wv, Wo, **_unused):
    x = np.asarray(x, dtype=np.float32)
    Wq = np.asarray(Wq, dtype=np.float32)
    Wk = np.asarray(Wk, dtype=np.float32)
    Wv = np.asarray(Wv, dtype=np.float32)
    Wo = np.asarray(Wo, dtype=np.float32)

    nc = _get_program()
    in_maps = _make_in_maps(x, Wq, Wk, Wv, Wo)
    res = run_bass_kernel_spmd(nc, in_maps, core_ids=list(range(N_CORES)))
    out = np.zeros((B, S, D), dtype=np.float64)
    for core in range(N_CORES):
        b = core // HG
        out[b] += res.results[core]["outT"].T.astype(np.float64)
    return out.astype(np.float32)


# revision 29
# speedup vs baseline: 1.0596x; 1.0082x over previous
"""Causal multi-head attention (B=2, S=2048, D=1024, H=16) on 8 TRN2 NeuronCores.

Sharding: batch*heads across cores. Core c handles batch c//4 and the 4 heads
g*4..g*4+3 where g = c%4. Weights are sliced per core (Megatron-style column
split of Wq/Wk/Wv, row split of Wo); each core produces a partial projected
output [D, S] (transposed) and the host sums the 4 partials per batch.

Everything on-chip is kept transposed ([feature, seq]) so no transposes are
ever needed on the forward path:
  qT/kT = wq/wk^T @ xT            (PE, fp16, contraction over D)
  v     = x @ Wv^T                (fp16; s on partitions, + ones col appended)
  sT    = k @ qT  [s_k=128, s_q]  (PE, contraction over dh=64, 2 heads packed
                                   via base-partition 0/64 row groups)
  eT    = exp(sT/8) -> fp16, then an on-chip 0/1 mask (built once with
          affine_select) zeroes the 128-wide causal band (GPSIMD)
  avB   = eT-subtile^T @ v_aug    [q=128, 65]  per (head, q-sub): q on
          PARTITIONS, so the matmul free size is 65, not 512 -- half the PE
          cycles of the [65, 512] orientation, and the softmax denominator
          (col 64) comes along free.
  o     = avB[:, 0:64] * recip(avB[:, 64]) broadcast along the FREE dim (DVE)
          -- no denominator-broadcast matmul needed at all.
  oT    = PE-transpose(o, identity)  [dims 128, q 128] per q-sub (128 cyc)
  partialT = wo^T-chunks @ oT     (PE, fp16, contraction over 256 head dims)

All 16-bit tensors are fp16 (not bf16): same PE/DMA cost, 4 more mantissa
bits. Partials stream out as fp16 (host sums in float64).

Scheduling: engines execute their instruction streams in emission order, so
the kernel is emitted as ONE interleaved stream. Attention chunks (latency-
bound on the PE->ACT->GPSIMD->PE chain) are diluted with fine-grained filler
units (2-4 matmuls each): QKV work for later tiles, deferred Wo work for
earlier tiles, and deferred per-phase normalization units (fins: fin_a =
den/recip/divide on ACT+DVE, fin_b = PE transposes + oT copy, popped 1 and 3
chunks into the next phase respectively so the PE never parks on the DVE
chain). A per-phase emission-time ledger of ACT-vs-PE nanoseconds pumps
fillers exactly when the scalar engine (exp) would fall behind; AV matmuls
trail their scores by AV_LAG chunks. The chunk-major prologue advances all
four k/q accumulators per arriving x-block pair. The final phase staggers
normalization per q-sub (q-sub s's AV accumulation closes at diag chunk
12+s), interleaving per-sub fins with the AV drain, then runs Wo u=0 across 8
borrowed PSUM slots, per-sub u=1 closes, and 4 per-sub output DMAs split
across the SP (HWDGE) and Pool (SWDGE) queues.
"""

from collections import deque

import numpy as np

import concourse.bass as bass
import concourse.mybir as mybir
import concourse.tile as tile
from concourse import bacc, masks
from concourse.bass_utils import run_bass_kernel_spmd

B = 2
S = 2048
D = 1024
H = 16
DH = 64
N_CORES = 8
HG = H // 4  # 4 heads per core
GM = 4 * DH  # 256 head dims per core
FP32 = mybir.dt.float32
F16 = mybir.dt.float16

S_TILE = 512  # q-tile width (PSUM bank)
N_ST = S // S_TILE  # 4
KC = 128  # k-chunk (partition dim of scoresT)
N_KC = S // KC  # 16
N_DC = D // 128  # 8 d-chunks
AV_LAG = 4  # chunks between scores and their AV matmuls (hides exp+mask latency)


def build_program():
    nc = bacc.Bacc("TRN2", target_bir_lowering=False, debug=False)

    xT = nc.dram_tensor("xT", [D, S], F16, kind="ExternalInput")
    wq = nc.dram_tensor("wq", [D, GM], F16, kind="ExternalInput")
    wk = nc.dram_tensor("wk", [D, GM], F16, kind="ExternalInput")
    wv = nc.dram_tensor("wv", [D, GM], F16, kind="ExternalInput")
    wo = nc.dram_tensor("wo", [GM, D], F16, kind="ExternalInput")
    outT = nc.dram_tensor("outT", [D, S], F16, kind="ExternalOutput")

    with tile.TileContext(nc) as tc:
        with (
            tc.tile_pool(name="persist", bufs=1) as persist,
            tc.tile_pool(name="xb", bufs=8) as xb_pool,
            tc.tile_pool(name="exp", bufs=10) as exp_pool,
            tc.tile_pool(name="small", bufs=4) as small_pool,
            tc.tile_pool(name="outsb", bufs=4) as out_pool,
            tc.tile_pool(name="mm", bufs=2, space="PSUM") as mm_pool,
            tc.tile_pool(name="scores", bufs=2, space="PSUM") as sc_pool,
            tc.tile_pool(name="av", bufs=2, space="PSUM") as av_pool,
        ):
            # ---- persistent SBUF tensors ----
            wo_sb = persist.tile([128, 2, D], F16, tag="wo")
            ones_col = persist.tile([128, 1], FP32, tag="ones")
            ident = persist.tile([128, 128], F16, tag="ident")
            w_sb = {}
            for name in ("q", "k", "v"):
                w_sb[name] = persist.tile(
                    [128, N_DC, GM], F16, tag=f"w{name}", name=f"w{name}sb"
                )
            # the very first wk chunk rides the Pool SWDGE queue, issued in
            # parallel with the SP (HWDGE) x stream: the first k matmul's
            # operands arrive ~1us sooner than a serial HWDGE head
            nc.gpsimd.dma_start(
                w_sb["k"][:, 0:1, :],
                wk.rearrange("(c p) m -> p c m", p=128)[:, 0:1, :],
            )
            nc.vector.memset(ones_col[:, :], 1.0)
            masks.make_identity(nc, ident[:, :])
            # causal mask patterns, generated on-chip (no DMA):
            # mask4[p, j, q] = 1.0 iff 128*j + p <= q  (memset on DVE: Pool's
            # startup time is the first-weight-DMA critical path)
            mask4 = persist.tile([128, 4, S_TILE], FP32, tag="mask4")
            nc.vector.memset(mask4[:, :, :], 1.0)
            for j in range(4):
                nc.gpsimd.affine_select(
                    mask4[:, j, :],
                    mask4[:, j, :],
                    pattern=[[1, S_TILE]],
                    compare_op=mybir.AluOpType.is_ge,
                    fill=0.0,
                    base=-128 * j,
                    channel_multiplier=-1,
                )

            qT = {}  # (u, t) -> [128, 512]   2 heads stacked (rows 0-63 / 64-127)
            kT = {}
            vt = {}  # c16 -> [128, HG, 65]   v chunk with ones col per head
            oT = {}  # (u, t) -> [128, 512]   attention out, dims on partitions
            for t in range(N_ST):
                for u in range(2):
                    qT[(u, t)] = persist.tile(
                        [128, S_TILE], F16, tag=f"qT{u}{t}", name=f"qT{u}{t}"
                    )
                    kT[(u, t)] = persist.tile(
                        [128, S_TILE], F16, tag=f"kT{u}{t}", name=f"kT{u}{t}"
                    )
                    oT[(u, t)] = persist.tile(
                        [128, S_TILE], F16, tag=f"oT{u}{t}", name=f"oT{u}{t}"
                    )
            for c16 in range(N_KC):
                vt[c16] = persist.tile(
                    [128, HG, DH + 1], F16, tag=f"v{c16}", name=f"v{c16}"
                )

            xb = {}

            def load_xb(t, c0, eng=None):
                # two d-chunks per DMA: halves the HWDGE issue cost (625ns
                # per DMA vs 364ns fp16 transfer, so issue rate dominates)
                blk = xb_pool.tile(
                    [128, 2, S_TILE], F16, tag="xb", name=f"xb{c0}_{t}"
                )
                (eng or nc.sync).dma_start(
                    blk[:, :, :],
                    xT[
                        c0 * 128 : (c0 + 2) * 128,
                        t * S_TILE : (t + 1) * S_TILE,
                    ].rearrange("(i p) m -> p i m", i=2),
                    )
                xb[(c0, t)] = blk[:, 0, :]
                xb[(c0 + 1, t)] = blk[:, 1, :]

            # ---- DMA stream order ----
            # tile-0 inputs first (weights in halves interleaved with x blocks
            # so the first k/q matmuls start ~3.5us in), then x tiles 1..3
            # stream ahead of their QKV filler units, then wo (needed late).
            wk_r = wk.rearrange("(c p) m -> p c m", p=128)
            wq_r = wq.rearrange("(c p) m -> p c m", p=128)
            wv_r = wv.rearrange("(c p) m -> p c m", p=128)
            # paced for the chunk-major prologue: per 2-chunk period, the k and
            # q weight chunks land just before the x block pair that uses them.
            # wk chunk 0 already rides the Pool queue, so SP leads with x.
            load_xb(0, 0)
            nc.sync.dma_start(w_sb["q"][:, 0:2, :], wq_r[:, 0:2, :])
            nc.sync.dma_start(w_sb["k"][:, 1:2, :], wk_r[:, 1:2, :])
            for p in range(1, 4):
                nc.sync.dma_start(
                    w_sb["k"][:, 2 * p : 2 * p + 2, :], wk_r[:, 2 * p : 2 * p + 2, :]
                )
                nc.sync.dma_start(
                    w_sb["q"][:, 2 * p : 2 * p + 2, :],
                    wq_r[:, 2 * p : 2 * p + 2, :],
                )
                load_xb(0, 2 * p)
            nc.sync.dma_start(w_sb["v"][:, 0:4, :], wv_r[:, 0:4, :])
            nc.sync.dma_start(w_sb["v"][:, 4:8, :], wv_r[:, 4:8, :])
            for c0 in range(0, N_DC, 2):
                load_xb(1, c0, eng=nc.gpsimd)
            nc.sync.dma_start(wo_sb[:, :, :], wo.rearrange("(u p) d -> p u d", p=128))
            for t in range(2, N_ST):
                for c0 in range(0, N_DC, 2):
                    load_xb(t, c0)

            # ---- emission thunks ----
            # Group PSUM tiles are created lazily by the first unit of each
            # group (cell dict) so mm_pool's buffer rotation follows actual
            # use order, not enqueue order.
            def qk_matmuls(name, u, t, cell, c0, c1):
                if c0 == 0:
                    cell["ps"] = mm_pool.tile(
                        [128, S_TILE], FP32, tag="mm", name=f"ps{name}{u}{t}"
                    )
                ps = cell["ps"]
                for c in range(c0, c1):
                    nc.tensor.matmul(
                        ps[:, :],
                        lhsT=w_sb[name][:, c, u * 128 : (u + 1) * 128],
                        rhs=xb[(c, t)][:, :],
                        start=(c == 0),
                        stop=(c == N_DC - 1),
                    )
                if c1 == N_DC:
                    dst = kT if name == "k" else qT
                    nc.vector.tensor_copy(dst[(u, t)][:, :], ps[:, :])

            def v_matmuls(t, s4, cell, c0, c1):
                c16 = 4 * t + s4
                if c0 == 0:
                    cell["ps"] = mm_pool.tile(
                        [128, GM], FP32, tag="mm", name=f"psv{c16}"
                    )
                ps = cell["ps"]
                for c in range(c0, c1):
                    nc.tensor.matmul(
                        ps[:, :],
                        lhsT=xb[(c, t)][:, s4 * 128 : (s4 + 1) * 128],
                        rhs=w_sb["v"][:, c, :],
                        start=(c == 0),
                        stop=(c == N_DC - 1),
                    )
                if c1 == N_DC:
                    nc.vector.tensor_copy(
                        vt[c16][:, :, 0:DH], ps.rearrange("p (h d) -> p h d", h=HG)
                    )
                    nc.gpsimd.tensor_copy(
                        vt[c16][:, :, DH : DH + 1],
                        ones_col[:, 0:1].broadcast_to((128, HG, 1)),
                    )

            def emit_wo_group(t, dc, on_act=False):
                po = mm_pool.tile([128, S_TILE], FP32, tag="mm", name=f"po{t}{dc}")
                for u in range(2):
                    nc.tensor.matmul(
                        po[:, :],
                        lhsT=wo_sb[:, u, dc * 128 : (dc + 1) * 128],
                        rhs=oT[(u, t)][:, :],
                        start=(u == 0),
                        stop=(u == 1),
                    )
                ob = out_pool.tile([128, S_TILE], F16, tag="ob")
                if on_act:  # tail: split the copy across ACT+DVE so the PSUM
                    # bank frees at PE pace, not copy pace
                    nc.scalar.copy(ob[:, 0 : S_TILE // 2], po[:, 0 : S_TILE // 2])
                    nc.vector.tensor_copy(ob[:, S_TILE // 2 :], po[:, S_TILE // 2 :])
                else:
                    nc.vector.tensor_copy(ob[:, :], po[:, :])
                nc.sync.dma_start(
                    outT[dc * 128 : (dc + 1) * 128, t * S_TILE : (t + 1) * S_TILE],
                    ob[:, :],
                )

            # filler queue: (tile_tag, cost_cycles, thunk). Attention chunks pump
            # filler units between chunks to keep the PE dense while exp/mask
            # latency elapses. QKV units are tagged with their tile (flushed
            # before that tile's attention); Wo units are tagged 99 (pump/drain
            # only -- they are enqueued once their oT inputs exist).
            fillers = deque()

            def enqueue_qkv_units(t):
                for name in ("k", "q"):
                    for u in range(2):
                        cell = {}
                        for c0 in range(0, N_DC, 2):
                            fillers.append(
                                (
                                    t,
                                    2 * S_TILE,
                                    lambda name=name, u=u, t=t, cell=cell, c0=c0: qk_matmuls(
                                        name, u, t, cell, c0, c0 + 2
                                    ),
                                )
                            )
                # v(t) is only consumed by (t,0)'s AV drain, so its units are
                # tagged t+0.5: they skip the phase-start flush and instead
                # pump as filler DURING (t,0), where late phases are starved
                for s4 in range(4):
                    cell = {}
                    for c0 in range(0, N_DC, 4):
                        fillers.append(
                            (
                                t + 0.5,
                                4 * GM,
                                lambda t=t, s4=s4, cell=cell, c0=c0: v_matmuls(
                                    t, s4, cell, c0, c0 + 4
                                ),
                            )
                        )

            # --- emission-time ACT-vs-PE ledger (reset per attention phase).
            # exp work accumulates act_ns; attention matmuls + pumped fillers
            # accumulate pe_ns. Pump fillers whenever ACT is ahead, so the PE
            # stream is diluted exactly where the scalar engine needs time.
            PE_CY = 1.0 / 2.4  # ns per cycle at peak
            SLACK = 900.0  # exp-pipeline fill depth: don't pump before ACT is
            # genuinely ahead of the PE stream in wall-clock terms
            ledger = {"pe": 0.0, "act": -SLACK}

            # deferred normalization units: (min_chunk, cost, thunk) popped by
            # pumps min_chunk chunks into the NEXT phase, when their upstream
            # ACT/DVE chain has drained
            fins = deque()
            phase_chunk = {"c": 0}

            def pump_ledger(max_tag, extra=0.0):
                ledger["act"] += extra
                while ledger["pe"] < ledger["act"] and (fins or fillers):
                    if fins and phase_chunk["c"] >= fins[0][0]:
                        entry = fins.popleft()
                        cost, thunk = entry[1], entry[2]
                    elif fillers:
                        entry = fillers[0]
                        tag, cost, thunk = entry[0], entry[1], entry[2]
                        if tag == 99:
                            wo_tile = entry[3]
                            if any(f[3] == wo_tile for f in fins):
                                # a wo(t) unit must never be emitted while a
                                # fin producing its oT input is still queued:
                                # the PE stream is in-order, so that would
                                # deadlock
                                return
                            if max_tag < N_ST:
                                # hold wo units for the filler-starved t3
                                # phases
                                return
                        elif tag > max_tag:
                            return
                        fillers.popleft()
                    else:
                        return
                    thunk()
                    ledger["pe"] += cost * PE_CY

            def flush_tile(t):
                # 99 = wo units (pump/drain only, reserved as late filler)
                while fillers and fillers[0][0] <= t:
                    _, _, thunk = fillers.popleft()
                    thunk()

            # prologue, chunk-major: all four k/q accumulation groups advance
            # per arriving x block (their PSUM lives in sc_pool, idle during
            # the prologue), so the PE has ~850ns of work per ~1.5us DMA chunk
            # instead of ~430ns. v groups become tile-0 fillers.
            sc0 = sc_pool.tile([128, 2 * S_TILE], FP32, tag="sc", name="pro_sc0")
            sc1 = sc_pool.tile([128, 2 * S_TILE], FP32, tag="sc", name="pro_sc1")
            pro = {("k", 0): sc0[:, 0:S_TILE], ("q", 0): sc0[:, S_TILE:],
                   ("k", 1): sc1[:, 0:S_TILE], ("q", 1): sc1[:, S_TILE:]}
            for s4 in range(4):
                cell = {}
                for c0 in range(0, N_DC, 4):
                    fillers.append(
                        (
                            0.5,
                            4 * GM,
                            lambda s4=s4, cell=cell, c0=c0: v_matmuls(
                                0, s4, cell, c0, c0 + 4
                            ),
                        )
                    )
            for c in range(N_DC):
                for name in ("k", "q"):
                    for u in range(2):
                        nc.tensor.matmul(
                            pro[(name, u)],
                            lhsT=w_sb[name][:, c, u * 128 : (u + 1) * 128],
                            rhs=xb[(c, 0)][:, :],
                            start=(c == 0),
                            stop=(c == N_DC - 1),
                        )
            for u in range(2):  # ACT is idle during the prologue; keep DVE free
                nc.scalar.copy(kT[(u, 0)][:, :], pro[("k", u)])
                nc.scalar.copy(qT[(u, 0)][:, :], pro[("q", u)])
            for t in range(1, N_ST):
                enqueue_qkv_units(t)

            for t in range(N_ST):
                nch = 4 * t + 4
                for hp in range(2):
                    u = hp
                    final = t == N_ST - 1 and hp == 1
                    if t > 0 or hp > 0:
                        # (t,0) needs kq u=0; (t,1) additionally needs u=1
                        flush_tile(t if hp == 0 else t + 0.3)
                    ledger["pe"] = 0.0
                    ledger["act"] = -SLACK
                    phase_chunk["c"] = 0
                    avs = [
                        av_pool.tile(
                            [128, 4, DH + 1], FP32, tag="av", name=f"av{t}{hp}{i}"
                        )
                        for i in range(2)
                    ]
                    pending_avs = deque()  # AV trails scores by AV_LAG chunks

                    def emit_av(cc, exx, jj, avs=avs, hp=hp, t=t, nch=nch):
                        # q-on-partitions AV: per (head, q-sub 128) the matmul
                        # free size is just 65 (64 dims + the ones/denominator
                        # col); q-sub s's accumulation closes at diag chunk
                        # 4t+s. PSUM accumulation-group starts are ZERO-REGION
                        # (2KB bank) granular: exactly ONE start and ONE stop
                        # per av tile -- the bank-wide pending-zero from the
                        # single start zeroes each sub-region on first touch.
                        s0 = max(jj, 0)
                        for i in range(2):
                            for s in range(s0, 4):
                                nc.tensor.matmul(
                                    avs[i][:, s, :],
                                    lhsT=exx[
                                        :,
                                        i * S_TILE + 128 * s : i * S_TILE
                                        + 128 * (s + 1),
                                    ],
                                    rhs=vt[cc][:, 2 * hp + i, :],
                                    start=(cc == 0 and s == s0),
                                    stop=(cc == nch - 1 and s == 3),
                                    skip_group_check=True,
                                )
                        ledger["pe"] += 2 * (4 - s0) * (DH + 1) * PE_CY

                    for c in range(nch):
                        # Diagonal chunks only touch q columns >= 128j
                        # (causal): scores / exp / AV skip the masked prefix.
                        j = c - 4 * t
                        q0 = 128 * j if j >= 0 else 0
                        w = S_TILE - q0
                        sc = sc_pool.tile([128, 2 * S_TILE], FP32, tag="sc")
                        for i in range(2):  # head parity: rows 0-63 / 64-127
                            bp = 64 * i
                            nc.tensor.matmul(
                                sc[:, i * S_TILE + q0 : (i + 1) * S_TILE],
                                lhsT=kT[(u, c // 4)][
                                    bp : bp + DH, (c % 4) * 128 : (c % 4 + 1) * 128
                                ],
                                rhs=qT[(u, t)][bp : bp + DH, q0:],
                                start=True,
                                stop=True,
                            )
                        ledger["pe"] += 2 * w * PE_CY
                        ex = exp_pool.tile([128, 2 * S_TILE], F16, tag="ex")
                        exv = ex.rearrange("p (i n) -> p i n", i=2)[:, :, q0:]
                        scv = sc.rearrange("p (i n) -> p i n", i=2)[:, :, q0:]
                        nc.scalar.activation(
                            exv, scv, mybir.ActivationFunctionType.Exp, scale=0.125
                        )
                        ledger["act"] += 2 * w * 0.833 + 242
                        if j >= 0:
                            # zero the causal triangle (mask is 0/1, exact).
                            # Only the band [q0, 128j+128) needs masking; one
                            # op covers both head slots via a broadcast mask.
                            bhi = 128 * j + 128
                            exb = ex.rearrange("p (i n) -> p i n", i=2)[
                                :, :, q0:bhi
                            ]
                            nc.gpsimd.tensor_mul(
                                exb,
                                exb,
                                mask4[:, j : j + 1, q0:bhi].broadcast_to(
                                    (128, 2, bhi - q0)
                                ),
                            )

                        pending_avs.append((c, ex, j))
                        phase_chunk["c"] = c
                        pump_ledger(t + 1)
                        if len(pending_avs) > AV_LAG:
                            emit_av(*pending_avs.popleft())
                    if hp == 0:
                        flush_tile(t + 0.5)  # v(t) needed by the AV drain

                    if not final:
                        while pending_avs:
                            emit_av(*pending_avs.popleft())
                            # the exp tail is still draining on ACT in
                            # wall-clock terms; keep filler between the drain
                            pump_ledger(t + 1, extra=700)
                        # normalization: rec = 1/den per (head, q-sub) on DVE,
                        # o = av * rec via FREE-dim broadcast (q is on
                        # partitions), then 4 PE transposes into oT. Split so
                        # the PE piece (fin_b) pops 3 chunks into the next
                        # phase, when fin_a's DVE chain has drained.
                        cell = {}

                        def emit_fin_a(avs=avs, cell=cell):
                            den = small_pool.tile([128, 2, 4], FP32, tag="den")
                            for i in range(2):
                                nc.vector.tensor_copy(den[:, i, :], avs[i][:, :, DH])
                            rec = small_pool.tile([128, 2, 4], FP32, tag="rec32")
                            nc.vector.reciprocal_approx_fast(
                                rec[:, :, :], den[:, :, :]
                            )
                            ob2 = small_pool.tile([128, 4, 128], F16, tag="obo")
                            for i in range(2):
                                nc.vector.tensor_mul(
                                    ob2[:, :, 64 * i : 64 * i + DH],
                                    avs[i][:, :, 0:DH],
                                    rec[:, i, :]
                                    .rearrange("p (a b) -> p a b", b=1)
                                    .broadcast_to((128, 4, DH)),
                                )
                            cell["ob2"] = ob2

                        def emit_fin_b(cell=cell, u=u, t=t):
                            # one mm_pool tile per transpose: each PE
                            # transpose is its own accumulation group and a
                            # group start claims a whole 2KB zero region, so
                            # outputs must not share a PSUM bank
                            ob2 = cell["ob2"]
                            for s in range(4):
                                pt = mm_pool.tile(
                                    [128, 128], F16, tag="mm", name=f"pt{t}{u}{s}"
                                )
                                nc.tensor.matmul(
                                    pt[:, :],
                                    lhsT=ob2[:, s, :],
                                    rhs=ident[:, :],
                                    is_transpose=True,
                                )
                                nc.vector.tensor_copy(
                                    oT[(u, t)][:, 128 * s : 128 * (s + 1)],
                                    pt[:, :],
                                )

                        fins.append((1, 0, emit_fin_a, t))
                        fins.append((3, 4 * 128, emit_fin_b, t))
                    else:
                        # ---- final phase: staggered per-q-sub normalization
                        # interleaved with the AV drain, then the Wo tail.
                        # Flush any queued fins (they produce oT(.,t3) that
                        # the tail's u0 matmuls read) and leftover wo units
                        # (their mm_pool allocations must all precede fpt's,
                        # or bufs=2 rotation would evict fpt mid-use).
                        while fins:
                            entry = fins.popleft()
                            entry[2]()
                            ledger["pe"] += entry[1] * PE_CY
                        while fillers:
                            entry = fillers.popleft()
                            entry[2]()
                            ledger["pe"] += entry[1] * PE_CY
                        fin_cells = {}

                        def fin_sub_a(s, avs=avs):
                            den = small_pool.tile(
                                [128, 2, 1], FP32, tag="den", name=f"fden{s}"
                            )
                            for i in range(2):
                                nc.scalar.copy(
                                    den[:, i, :], avs[i][:, s, DH : DH + 1]
                                )
                            rec = small_pool.tile(
                                [128, 2, 1], FP32, tag="rec32", name=f"frec{s}"
                            )
                            nc.vector.reciprocal_approx_fast(
                                rec[:, :, :], den[:, :, :]
                            )
                            ob2 = small_pool.tile(
                                [128, 1, 128], F16, tag="obo", name=f"fob{s}"
                            )
                            for i in range(2):
                                nc.vector.tensor_mul(
                                    ob2[:, 0, 64 * i : 64 * i + DH],
                                    avs[i][:, s, 0:DH],
                                    rec[:, i, :].broadcast_to((128, DH)),
                                )
                            fin_cells[s] = ob2

                        def fin_sub_b(s, u=u, t=t):
                            ob2 = fin_cells[s]
                            pt = mm_pool.tile(
                                [128, 128], F16, tag="mm", name=f"fpt{s}"
                            )
                            nc.tensor.matmul(
                                pt[:, :],
                                lhsT=ob2[:, 0, :],
                                rhs=ident[:, :],
                                is_transpose=True,
                            )
                            nc.vector.tensor_copy(
                                oT[(u, t)][:, 128 * s : 128 * (s + 1)],
                                pt[:, :],
                            )

                        while pending_avs:
                            cc, exx, jj = pending_avs.popleft()
                            emit_av(cc, exx, jj)
                            s_stop = cc - 4 * t
                            if s_stop >= 0:
                                fin_sub_a(s_stop)
                                if s_stop >= 1:
                                    fin_sub_b(s_stop - 1)
                            pump_ledger(t + 1, extra=700)

                        # kernel tail: last tile's 8 Wo groups across 8
                        # borrowed PSUM slots. u=0 (full width, needs only
                        # oT(0,t3)) runs while the staggered fin chains drain;
                        # u=1 closes per q-sub as its oT slice lands; outputs
                        # leave as 4 per-sub DMAs split across SP and Pool
                        # queues so no issue serialization at the very end.
                        tl = N_ST - 1
                        tail_a = sc_pool.tile(
                            [128, 2 * S_TILE], FP32, tag="sc", name="tail_a"
                        )
                        tail_b = sc_pool.tile(
                            [128, 2 * S_TILE], FP32, tag="sc", name="tail_b"
                        )
                        slots = [
                            tail_a[:, 0:S_TILE],
                            tail_a[:, S_TILE:],
                            tail_b[:, 0:S_TILE],
                            tail_b[:, S_TILE:],
                            None,  # mm slots allocated after fin_sub_b(3)
                            None,
                            None,  # av slots borrowed once avs are dead
                            None,
                        ]

                        def wo_u0(dc):
                            nc.tensor.matmul(
                                slots[dc],
                                lhsT=wo_sb[:, 0, dc * 128 : (dc + 1) * 128],
                                rhs=oT[(0, tl)][:, :],
                                start=True,
                                stop=False,
                                skip_group_check=True,
                            )

                        def wo_u1(s, dc):
                            nc.tensor.matmul(
                                slots[dc][:, 128 * s : 128 * (s + 1)],
                                lhsT=wo_sb[:, 1, dc * 128 : (dc + 1) * 128],
                                rhs=oT[(1, tl)][:, 128 * s : 128 * (s + 1)],
                                start=False,
                                stop=(s == 3),
                                skip_group_check=True,
                            )

                        # outputs leave as two half-tile DMAs [128, 8, 256]
                        # (512B descriptors -- no small-elem latency penalty)
                        # on separate queues. Half-1 (subs 0-1, whose fins
                        # close early in the drain) is fully emitted per-dc
                        # right after u0 so its transfer overlaps the rest of
                        # the tail; half-2 closes per-dc behind it.
                        obh = {}
                        for h in range(2):
                            obh[h] = out_pool.tile(
                                [128, N_DC, 256], F16, tag="ob", name=f"obh{h}"
                            )

                        def half_copy(h, dc):
                            # dc 0-3 live in the two sc tiles as column pairs:
                            # one strided copy moves both dcs of a tile
                            if dc < 4:
                                if dc % 2 == 1:
                                    return
                                tile_ab = tail_a if dc == 0 else tail_b
                                src = tile_ab.rearrange("p (i m) -> p i m", i=2)[
                                    :, :, 256 * h : 256 * (h + 1)
                                ]
                                dsth = obh[h][:, dc : dc + 2, :]
                                if dc == 0:
                                    nc.scalar.copy(dsth, src)
                                else:
                                    nc.vector.tensor_copy(dsth, src)
                                return
                            src = slots[dc][:, 256 * h : 256 * (h + 1)]
                            dsth = obh[h][:, dc, :]
                            if dc % 2 == 0:
                                nc.scalar.copy(dsth, src)
                            else:
                                nc.vector.tensor_copy(dsth, src)

                        for dc in range(4):
                            wo_u0(dc)
                            wo_u1(0, dc)
                            wo_u1(1, dc)
                            if dc % 2 == 1:
                                half_copy(0, dc - 1)
                        fin_sub_b(3)  # before mm tail allocs (rotation safety)
                        slots[4] = mm_pool.tile(
                            [128, S_TILE], FP32, tag="mm", name="tail_m0"
                        )[:, :]
                        slots[5] = mm_pool.tile(
                            [128, S_TILE], FP32, tag="mm", name="tail_m1"
                        )[:, :]
                        slots[6] = av_pool.tile(
                            [128, S_TILE], FP32, tag="av", name="tail_v0"
                        )[:, :]
                        slots[7] = av_pool.tile(
                            [128, S_TILE], FP32, tag="av", name="tail_v1"
                        )[:, :]
                        for dc in range(4, 8):
                            wo_u0(dc)
                            wo_u1(0, dc)
                            wo_u1(1, dc)
                            half_copy(0, dc)
                        nc.sync.dma_start(
                            outT[:, tl * S_TILE : tl * S_TILE + 256].rearrange(
                                "(i p) m -> p i m", i=N_DC
                            ),
                            obh[0][:, :, :],
                        )
                        # subs 2-3 close dc-major and leave as two quarter
                        # pieces so the very last transfer is only 728ns
                        for dc in range(4):
                            wo_u1(2, dc)
                            wo_u1(3, dc)
                            if dc % 2 == 1:
                                half_copy(1, dc - 1)
                        nc.sync.dma_start(
                            outT[
                                0 : 4 * 128, tl * S_TILE + 256 : tl * S_TILE + 512
                            ].rearrange("(i p) m -> p i m", i=4),
                            obh[1][:, 0:4, :],
                        )
                        for dc in range(4, 8):
                            wo_u1(2, dc)
                            wo_u1(3, dc)
                            half_copy(1, dc)
                        nc.sync.dma_start(
                            outT[
                                4 * 128 :, tl * S_TILE + 256 : tl * S_TILE + 512
                            ].rearrange("(i p) m -> p i m", i=4),
                            obh[1][:, 4:8, :],
                        )
                # Wo for this tile becomes filler work for later attention
                # (the last tile's Wo is the kernel tail, emitted above)
                if t < N_ST - 1:
                    for dc in range(N_DC):
                        fillers.append(
                            (
                                99,
                                2 * S_TILE,
                                lambda t=t, dc=dc: emit_wo_group(t, dc),
                                t,
                            )
                        )
            while fillers:
                fillers.popleft()[2]()
            while fins:
                fins.popleft()[2]()
    nc.compile()
    return nc


_NC_CACHE = None


def _get_program():
    global _NC_CACHE
    if _NC_CACHE is None:
        _NC_CACHE = build_program()
    return _NC_CACHE


def _make_in_maps(x, Wq, Wk, Wv, Wo):
    f16 = np.float16
    xTs = [np.ascontiguousarray(x[b].T).astype(f16) for b in range(B)]
    in_maps = []
    for core in range(N_CORES):
        b, g = divmod(core, HG)
        r0, r1 = g * GM, (g + 1) * GM
        in_maps.append(
            {
                "xT": xTs[b],
                "wq": np.ascontiguousarray(Wq[r0:r1, :].T).astype(f16),
                "wk": np.ascontiguousarray(Wk[r0:r1, :].T).astype(f16),
                "wv": np.ascontiguousarray(Wv[r0:r1, :].T).astype(f16),
                "wo": np.ascontiguousarray(Wo[:, r0:r1].T).astype(f16),
            }
        )
    return in_maps


def kernel(x, Wq, Wk, Wv, Wo, **_unused):
    x = np.asarray(x, dtype=np.float32)
    Wq = np.asarray(Wq, dtype=np.float32)
    Wk = np.asarray(Wk, dtype=np.float32)
    Wv = np.asarray(Wv, dtype=np.float32)
    Wo = np.asarray(Wo, dtype=np.float32)

    nc = _get_program()
    in_maps = _make_in_maps(x, Wq, Wk, Wv, Wo)
    res = run_bass_kernel_spmd(nc, in_maps, core_ids=list(range(N_CORES)))
    out = np.zeros((B, S, D), dtype=np.float64)
    for core in range(N_CORES):
        b = core // HG
        out[b] += res.results[core]["outT"].T.astype(np.float64)
    return out.astype(np.float32)


# revision 32
# speedup vs baseline: 1.0754x; 1.0149x over previous
"""Causal multi-head attention (B=2, S=2048, D=1024, H=16) on 8 TRN2 NeuronCores.

Sharding: batch*heads across cores. Core c handles batch c//4 and the 4 heads
g*4..g*4+3 where g = c%4. Weights are sliced per core (Megatron-style column
split of Wq/Wk/Wv, row split of Wo); each core produces a partial projected
output [D, S] (transposed) and the host sums the 4 partials per batch.

Everything on-chip is kept transposed ([feature, seq]) so no transposes are
ever needed on the forward path:
  qT/kT = wq/wk^T @ xT            (PE, fp16, contraction over D)
  v     = x @ Wv^T                (fp16; s on partitions, + ones col appended)
  sT    = k @ qT  [s_k=128, s_q]  (PE, contraction over dh=64, 2 heads packed
                                   via base-partition 0/64 row groups)
  eT    = exp(sT/8) -> fp16, then an on-chip 0/1 mask (built once with
          affine_select) zeroes the 128-wide causal band (GPSIMD)
  avB   = eT-subtile^T @ v_aug    [q=128, 65]  per (head, q-sub): q on
          PARTITIONS, so the matmul free size is 65, not 512 -- half the PE
          cycles of the [65, 512] orientation, and the softmax denominator
          (col 64) comes along free.
  o     = avB[:, 0:64] * recip(avB[:, 64]) broadcast along the FREE dim (DVE)
          -- no denominator-broadcast matmul needed at all.
  oT    = PE-transpose(o, identity)  [dims 128, q 128] per q-sub (128 cyc)
  partialT = wo^T-chunks @ oT     (PE, fp16, contraction over 256 head dims)

All 16-bit tensors are fp16 (not bf16): same PE/DMA cost, 4 more mantissa
bits. Partials stream out as fp16 (host sums in float64).

Scheduling: engines execute their instruction streams in emission order, so
the kernel is emitted as ONE interleaved stream. Attention chunks (latency-
bound on the PE->ACT->GPSIMD->PE chain) are diluted with fine-grained filler
units (2-4 matmuls each): QKV work for later tiles, deferred Wo work for
earlier tiles, and deferred per-phase normalization units (fins: fin_a =
den/recip/divide on ACT+DVE, fin_b = PE transposes + oT copy, popped 1 and 3
chunks into the next phase respectively so the PE never parks on the DVE
chain). A per-phase emission-time ledger of ACT-vs-PE nanoseconds pumps
fillers exactly when the scalar engine (exp) would fall behind; AV matmuls
trail their scores by AV_LAG chunks. The chunk-major prologue advances all
four k/q accumulators per arriving x-block pair. The final phase staggers
normalization per q-sub (q-sub s's AV accumulation closes at diag chunk
12+s), interleaving per-sub fins with the AV drain, then runs Wo u=0 across 8
borrowed PSUM slots, per-sub u=1 closes, and 4 per-sub output DMAs split
across the SP (HWDGE) and Pool (SWDGE) queues.
"""

from collections import deque

import numpy as np

import concourse.bass as bass
import concourse.mybir as mybir
import concourse.tile as tile
from concourse import bacc, masks
from concourse.bass_utils import run_bass_kernel_spmd

B = 2
S = 2048
D = 1024
H = 16
DH = 64
N_CORES = 8
HG = H // 4  # 4 heads per core
GM = 4 * DH  # 256 head dims per core
FP32 = mybir.dt.float32
F16 = mybir.dt.float16

S_TILE = 512  # q-tile width (PSUM bank)
N_ST = S // S_TILE  # 4
KC = 128  # k-chunk (partition dim of scoresT)
N_KC = S // KC  # 16
N_DC = D // 128  # 8 d-chunks
AV_LAG = 4  # chunks between scores and their AV matmuls (hides exp+mask latency)


def build_program():
    nc = bacc.Bacc("TRN2", target_bir_lowering=False, debug=False)

    xT = nc.dram_tensor("xT", [D, S], F16, kind="ExternalInput")
    wq = nc.dram_tensor("wq", [D, GM], F16, kind="ExternalInput")
    wk = nc.dram_tensor("wk", [D, GM], F16, kind="ExternalInput")
    wv = nc.dram_tensor("wv", [D, GM], F16, kind="ExternalInput")
    wo = nc.dram_tensor("wo", [GM, D], F16, kind="ExternalInput")
    outT = nc.dram_tensor("outT", [D, S], F16, kind="ExternalOutput")

    with tile.TileContext(nc) as tc:
        with (
            tc.tile_pool(name="persist", bufs=1) as persist,
            tc.tile_pool(name="xb", bufs=8) as xb_pool,
            tc.tile_pool(name="exp", bufs=10) as exp_pool,
            tc.tile_pool(name="small", bufs=4) as small_pool,
            tc.tile_pool(name="outsb", bufs=4) as out_pool,
            tc.tile_pool(name="mm", bufs=2, space="PSUM") as mm_pool,
            tc.tile_pool(name="scores", bufs=2, space="PSUM") as sc_pool,
            tc.tile_pool(name="av", bufs=2, space="PSUM") as av_pool,
        ):
            # ---- persistent SBUF tensors ----
            wo_sb = persist.tile([128, 2, D], F16, tag="wo")
            ones_col = persist.tile([128, 1], FP32, tag="ones")
            ident = persist.tile([128, 128], F16, tag="ident")
            w_sb = {}
            for name in ("q", "k", "v"):
                w_sb[name] = persist.tile(
                    [128, N_DC, GM], F16, tag=f"w{name}", name=f"w{name}sb"
                )
            # the very first wk chunk rides the Pool SWDGE queue, issued in
            # parallel with the SP (HWDGE) x stream: the first k matmul's
            # operands arrive ~1us sooner than a serial HWDGE head
            nc.gpsimd.dma_start(
                w_sb["k"][:, 0:1, :],
                wk.rearrange("(c p) m -> p c m", p=128)[:, 0:1, :],
            )
            nc.vector.memset(ones_col[:, :], 1.0)
            masks.make_identity(nc, ident[:, :])
            # causal mask patterns, generated on-chip (no DMA):
            # mask4[p, j, q] = 1.0 iff 128*j + p <= q  (memset on DVE: Pool's
            # startup time is the first-weight-DMA critical path)
            mask4 = persist.tile([128, 4, S_TILE], FP32, tag="mask4")
            nc.vector.memset(mask4[:, :, :], 1.0)
            for j in range(4):
                nc.gpsimd.affine_select(
                    mask4[:, j, :],
                    mask4[:, j, :],
                    pattern=[[1, S_TILE]],
                    compare_op=mybir.AluOpType.is_ge,
                    fill=0.0,
                    base=-128 * j,
                    channel_multiplier=-1,
                )

            qT = {}  # (u, t) -> [128, 512]   2 heads stacked (rows 0-63 / 64-127)
            kT = {}
            vt = {}  # c16 -> [128, HG, 65]   v chunk with ones col per head
            oT = {}  # (u, t) -> [128, 512]   attention out, dims on partitions
            for t in range(N_ST):
                for u in range(2):
                    qT[(u, t)] = persist.tile(
                        [128, S_TILE], F16, tag=f"qT{u}{t}", name=f"qT{u}{t}"
                    )
                    kT[(u, t)] = persist.tile(
                        [128, S_TILE], F16, tag=f"kT{u}{t}", name=f"kT{u}{t}"
                    )
                    oT[(u, t)] = persist.tile(
                        [128, S_TILE], F16, tag=f"oT{u}{t}", name=f"oT{u}{t}"
                    )
            for c16 in range(N_KC):
                vt[c16] = persist.tile(
                    [128, HG, DH + 1], F16, tag=f"v{c16}", name=f"v{c16}"
                )

            xb = {}

            def load_xb(t, c0, eng=None):
                # two d-chunks per DMA: halves the HWDGE issue cost (625ns
                # per DMA vs 364ns fp16 transfer, so issue rate dominates)
                blk = xb_pool.tile(
                    [128, 2, S_TILE], F16, tag="xb", name=f"xb{c0}_{t}"
                )
                (eng or nc.sync).dma_start(
                    blk[:, :, :],
                    xT[
                        c0 * 128 : (c0 + 2) * 128,
                        t * S_TILE : (t + 1) * S_TILE,
                    ].rearrange("(i p) m -> p i m", i=2),
                    )
                xb[(c0, t)] = blk[:, 0, :]
                xb[(c0 + 1, t)] = blk[:, 1, :]

            # ---- DMA stream order ----
            # tile-0 inputs first (weights in halves interleaved with x blocks
            # so the first k/q matmuls start ~3.5us in), then x tiles 1..3
            # stream ahead of their QKV filler units, then wo (needed late).
            wk_r = wk.rearrange("(c p) m -> p c m", p=128)
            wq_r = wq.rearrange("(c p) m -> p c m", p=128)
            wv_r = wv.rearrange("(c p) m -> p c m", p=128)
            # paced for the chunk-major prologue: per 2-chunk period, the k and
            # q weight chunks land just before the x block pair that uses them.
            # wk chunk 0 already rides the Pool queue, so SP leads with x.
            load_xb(0, 0)
            load_xb(0, 2, eng=nc.gpsimd)
            nc.sync.dma_start(w_sb["q"][:, 0:2, :], wq_r[:, 0:2, :])
            nc.sync.dma_start(w_sb["k"][:, 1:4, :], wk_r[:, 1:4, :])
            load_xb(0, 4)
            load_xb(0, 6, eng=nc.gpsimd)
            nc.sync.dma_start(w_sb["q"][:, 2:4, :], wq_r[:, 2:4, :])
            nc.sync.dma_start(w_sb["k"][:, 4:8, :], wk_r[:, 4:8, :])
            nc.sync.dma_start(w_sb["q"][:, 4:8, :], wq_r[:, 4:8, :])
            nc.sync.dma_start(w_sb["v"][:, 0:4, :], wv_r[:, 0:4, :])
            nc.sync.dma_start(w_sb["v"][:, 4:8, :], wv_r[:, 4:8, :])
            for c0 in range(0, N_DC, 2):
                load_xb(1, c0, eng=nc.gpsimd)
            nc.sync.dma_start(wo_sb[:, :, :], wo.rearrange("(u p) d -> p u d", p=128))
            for t in range(2, N_ST):
                for c0 in range(0, N_DC, 2):
                    load_xb(t, c0)

            # ---- emission thunks ----
            # Group PSUM tiles are created lazily by the first unit of each
            # group (cell dict) so mm_pool's buffer rotation follows actual
            # use order, not enqueue order.
            def qk_matmuls(name, u, t, cell, c0, c1):
                if c0 == 0:
                    cell["ps"] = mm_pool.tile(
                        [128, S_TILE], FP32, tag="mm", name=f"ps{name}{u}{t}"
                    )
                ps = cell["ps"]
                for c in range(c0, c1):
                    nc.tensor.matmul(
                        ps[:, :],
                        lhsT=w_sb[name][:, c, u * 128 : (u + 1) * 128],
                        rhs=xb[(c, t)][:, :],
                        start=(c == 0),
                        stop=(c == N_DC - 1),
                    )
                if c1 == N_DC:
                    dst = kT if name == "k" else qT
                    nc.vector.tensor_copy(dst[(u, t)][:, :], ps[:, :])

            def v_matmuls(t, s4, cell, c0, c1):
                c16 = 4 * t + s4
                if c0 == 0:
                    cell["ps"] = mm_pool.tile(
                        [128, GM], FP32, tag="mm", name=f"psv{c16}"
                    )
                ps = cell["ps"]
                for c in range(c0, c1):
                    nc.tensor.matmul(
                        ps[:, :],
                        lhsT=xb[(c, t)][:, s4 * 128 : (s4 + 1) * 128],
                        rhs=w_sb["v"][:, c, :],
                        start=(c == 0),
                        stop=(c == N_DC - 1),
                    )
                if c1 == N_DC:
                    nc.vector.tensor_copy(
                        vt[c16][:, :, 0:DH], ps.rearrange("p (h d) -> p h d", h=HG)
                    )
                    nc.gpsimd.tensor_copy(
                        vt[c16][:, :, DH : DH + 1],
                        ones_col[:, 0:1].broadcast_to((128, HG, 1)),
                    )

            def emit_wo_group(t, dc, on_act=False):
                po = mm_pool.tile([128, S_TILE], FP32, tag="mm", name=f"po{t}{dc}")
                for u in range(2):
                    nc.tensor.matmul(
                        po[:, :],
                        lhsT=wo_sb[:, u, dc * 128 : (dc + 1) * 128],
                        rhs=oT[(u, t)][:, :],
                        start=(u == 0),
                        stop=(u == 1),
                    )
                ob = out_pool.tile([128, S_TILE], F16, tag="ob")
                if on_act:  # tail: split the copy across ACT+DVE so the PSUM
                    # bank frees at PE pace, not copy pace
                    nc.scalar.copy(ob[:, 0 : S_TILE // 2], po[:, 0 : S_TILE // 2])
                    nc.vector.tensor_copy(ob[:, S_TILE // 2 :], po[:, S_TILE // 2 :])
                else:
                    nc.vector.tensor_copy(ob[:, :], po[:, :])
                nc.sync.dma_start(
                    outT[dc * 128 : (dc + 1) * 128, t * S_TILE : (t + 1) * S_TILE],
                    ob[:, :],
                )

            # filler queue: (tile_tag, cost_cycles, thunk). Attention chunks pump
            # filler units between chunks to keep the PE dense while exp/mask
            # latency elapses. QKV units are tagged with their tile (flushed
            # before that tile's attention); Wo units are tagged 99 (pump/drain
            # only -- they are enqueued once their oT inputs exist).
            fillers = deque()

            def enqueue_qkv_units(t):
                for name in ("k", "q"):
                    for u in range(2):
                        cell = {}
                        for c0 in range(0, N_DC, 2):
                            fillers.append(
                                (
                                    t,
                                    2 * S_TILE,
                                    lambda name=name, u=u, t=t, cell=cell, c0=c0: qk_matmuls(
                                        name, u, t, cell, c0, c0 + 2
                                    ),
                                )
                            )
                # v(t) is only consumed by (t,0)'s AV drain, so its units are
                # tagged t+0.5: they skip the phase-start flush and instead
                # pump as filler DURING (t,0), where late phases are starved
                for s4 in range(4):
                    cell = {}
                    for c0 in range(0, N_DC, 4):
                        fillers.append(
                            (
                                t + 0.5,
                                4 * GM,
                                lambda t=t, s4=s4, cell=cell, c0=c0: v_matmuls(
                                    t, s4, cell, c0, c0 + 4
                                ),
                            )
                        )

            # --- emission-time ACT-vs-PE ledger (reset per attention phase).
            # exp work accumulates act_ns; attention matmuls + pumped fillers
            # accumulate pe_ns. Pump fillers whenever ACT is ahead, so the PE
            # stream is diluted exactly where the scalar engine needs time.
            PE_CY = 1.0 / 2.4  # ns per cycle at peak
            SLACK = 900.0  # exp-pipeline fill depth: don't pump before ACT is
            # genuinely ahead of the PE stream in wall-clock terms
            ledger = {"pe": 0.0, "act": -SLACK}

            # deferred normalization units: (min_chunk, cost, thunk) popped by
            # pumps min_chunk chunks into the NEXT phase, when their upstream
            # ACT/DVE chain has drained
            fins = deque()
            phase_chunk = {"c": 0}

            def pump_ledger(max_tag, extra=0.0):
                ledger["act"] += extra
                while ledger["pe"] < ledger["act"] and (fins or fillers):
                    if fins and phase_chunk["c"] >= fins[0][0]:
                        entry = fins.popleft()
                        cost, thunk = entry[1], entry[2]
                    elif fillers:
                        entry = fillers[0]
                        tag, cost, thunk = entry[0], entry[1], entry[2]
                        if tag == 99:
                            wo_tile = entry[3]
                            if any(f[3] == wo_tile for f in fins):
                                # a wo(t) unit must never be emitted while a
                                # fin producing its oT input is still queued:
                                # the PE stream is in-order, so that would
                                # deadlock
                                return
                            if max_tag < N_ST:
                                # hold wo units for the filler-starved t3
                                # phases
                                return
                        elif tag > max_tag:
                            return
                        fillers.popleft()
                    else:
                        return
                    thunk()
                    ledger["pe"] += cost * PE_CY

            def flush_tile(t):
                # 99 = wo units (pump/drain only, reserved as late filler)
                while fillers and fillers[0][0] <= t:
                    _, _, thunk = fillers.popleft()
                    thunk()

            # prologue, chunk-major: all four k/q accumulation groups advance
            # per arriving x block (their PSUM lives in sc_pool, idle during
            # the prologue), so the PE has ~850ns of work per ~1.5us DMA chunk
            # instead of ~430ns. v groups become tile-0 fillers.
            sc0 = sc_pool.tile([128, 2 * S_TILE], FP32, tag="sc", name="pro_sc0")
            sc1 = sc_pool.tile([128, 2 * S_TILE], FP32, tag="sc", name="pro_sc1")
            pro = {("k", 0): sc0[:, 0:S_TILE], ("q", 0): sc0[:, S_TILE:],
                   ("k", 1): sc1[:, 0:S_TILE], ("q", 1): sc1[:, S_TILE:]}
            for s4 in range(4):
                cell = {}
                for c0 in range(0, N_DC, 4):
                    fillers.append(
                        (
                            0.5,
                            4 * GM,
                            lambda s4=s4, cell=cell, c0=c0: v_matmuls(
                                0, s4, cell, c0, c0 + 4
                            ),
                        )
                    )
            for c in range(N_DC):
                for name in ("k", "q"):
                    for u in range(2):
                        nc.tensor.matmul(
                            pro[(name, u)],
                            lhsT=w_sb[name][:, c, u * 128 : (u + 1) * 128],
                            rhs=xb[(c, 0)][:, :],
                            start=(c == 0),
                            stop=(c == N_DC - 1),
                        )
            for u in range(2):  # ACT is idle during the prologue; keep DVE free
                nc.scalar.copy(kT[(u, 0)][:, :], pro[("k", u)])
                nc.scalar.copy(qT[(u, 0)][:, :], pro[("q", u)])
            for t in range(1, N_ST):
                enqueue_qkv_units(t)

            for t in range(N_ST):
                nch = 4 * t + 4
                for hp in range(2):
                    u = hp
                    final = t == N_ST - 1 and hp == 1
                    if t > 0 or hp > 0:
                        # (t,0) needs kq u=0; (t,1) additionally needs u=1
                        flush_tile(t if hp == 0 else t + 0.3)
                    ledger["pe"] = 0.0
                    ledger["act"] = -SLACK
                    phase_chunk["c"] = 0
                    avs = [
                        av_pool.tile(
                            [128, 4, DH + 1], FP32, tag="av", name=f"av{t}{hp}{i}"
                        )
                        for i in range(2)
                    ]
                    pending_avs = deque()  # AV trails scores by AV_LAG chunks

                    def emit_av(cc, exx, jj, avs=avs, hp=hp, t=t, nch=nch):
                        # q-on-partitions AV: per (head, q-sub 128) the matmul
                        # free size is just 65 (64 dims + the ones/denominator
                        # col); q-sub s's accumulation closes at diag chunk
                        # 4t+s. PSUM accumulation-group starts are ZERO-REGION
                        # (2KB bank) granular: exactly ONE start and ONE stop
                        # per av tile -- the bank-wide pending-zero from the
                        # single start zeroes each sub-region on first touch.
                        s0 = max(jj, 0)
                        for i in range(2):
                            for s in range(s0, 4):
                                nc.tensor.matmul(
                                    avs[i][:, s, :],
                                    lhsT=exx[
                                        :,
                                        i * S_TILE + 128 * s : i * S_TILE
                                        + 128 * (s + 1),
                                    ],
                                    rhs=vt[cc][:, 2 * hp + i, :],
                                    start=(cc == 0 and s == s0),
                                    stop=(cc == nch - 1 and s == 3),
                                    skip_group_check=True,
                                )
                        ledger["pe"] += 2 * (4 - s0) * (DH + 1) * PE_CY

                    for c in range(nch):
                        # Diagonal chunks only touch q columns >= 128j
                        # (causal): scores / exp / AV skip the masked prefix.
                        j = c - 4 * t
                        q0 = 128 * j if j >= 0 else 0
                        w = S_TILE - q0
                        sc = sc_pool.tile([128, 2 * S_TILE], FP32, tag="sc")
                        for i in range(2):  # head parity: rows 0-63 / 64-127
                            bp = 64 * i
                            nc.tensor.matmul(
                                sc[:, i * S_TILE + q0 : (i + 1) * S_TILE],
                                lhsT=kT[(u, c // 4)][
                                    bp : bp + DH, (c % 4) * 128 : (c % 4 + 1) * 128
                                ],
                                rhs=qT[(u, t)][bp : bp + DH, q0:],
                                start=True,
                                stop=True,
                            )
                        ledger["pe"] += 2 * w * PE_CY
                        ex = exp_pool.tile([128, 2 * S_TILE], F16, tag="ex")
                        exv = ex.rearrange("p (i n) -> p i n", i=2)[:, :, q0:]
                        scv = sc.rearrange("p (i n) -> p i n", i=2)[:, :, q0:]
                        nc.scalar.activation(
                            exv, scv, mybir.ActivationFunctionType.Exp, scale=0.125
                        )
                        ledger["act"] += 2 * w * 0.833 + 242
                        if j >= 0:
                            # zero the causal triangle (mask is 0/1, exact).
                            # Only the band [q0, 128j+128) needs masking; one
                            # op covers both head slots via a broadcast mask.
                            bhi = 128 * j + 128
                            exb = ex.rearrange("p (i n) -> p i n", i=2)[
                                :, :, q0:bhi
                            ]
                            nc.gpsimd.tensor_mul(
                                exb,
                                exb,
                                mask4[:, j : j + 1, q0:bhi].broadcast_to(
                                    (128, 2, bhi - q0)
                                ),
                            )

                        pending_avs.append((c, ex, j))
                        phase_chunk["c"] = c
                        pump_ledger(t + 1)
                        if len(pending_avs) > AV_LAG:
                            emit_av(*pending_avs.popleft())
                    if hp == 0:
                        flush_tile(t + 0.5)  # v(t) needed by the AV drain

                    if not final:
                        while pending_avs:
                            emit_av(*pending_avs.popleft())
                            # the exp tail is still draining on ACT in
                            # wall-clock terms; keep filler between the drain
                            pump_ledger(t + 1, extra=400)
                        # normalization: rec = 1/den per (head, q-sub) on DVE,
                        # o = av * rec via FREE-dim broadcast (q is on
                        # partitions), then 4 PE transposes into oT. Split so
                        # the PE piece (fin_b) pops 3 chunks into the next
                        # phase, when fin_a's DVE chain has drained.
                        cell = {}

                        def emit_fin_a(avs=avs, cell=cell):
                            den = small_pool.tile([128, 2, 4], FP32, tag="den")
                            for i in range(2):
                                nc.vector.tensor_copy(den[:, i, :], avs[i][:, :, DH])
                            rec = small_pool.tile([128, 2, 4], FP32, tag="rec32")
                            nc.vector.reciprocal_approx_fast(
                                rec[:, :, :], den[:, :, :]
                            )
                            ob2 = small_pool.tile([128, 4, 128], F16, tag="obo")
                            for i in range(2):
                                nc.vector.tensor_mul(
                                    ob2[:, :, 64 * i : 64 * i + DH],
                                    avs[i][:, :, 0:DH],
                                    rec[:, i, :]
                                    .rearrange("p (a b) -> p a b", b=1)
                                    .broadcast_to((128, 4, DH)),
                                )
                            cell["ob2"] = ob2

                        def emit_fin_b(cell=cell, u=u, t=t):
                            # all 4 transposes share ONE PSUM bank: a single
                            # group start (pending-zero covers the bank, each
                            # sub-region zeroes on first touch) avoids 4x
                            # mm_pool rotation and 4 small copies
                            ob2 = cell["ob2"]
                            pt = mm_pool.tile(
                                [128, 4, 128], F16, tag="mm", name=f"pt{t}{u}"
                            )
                            for s in range(4):
                                nc.tensor.matmul(
                                    pt[:, s, :],
                                    lhsT=ob2[:, s, :],
                                    rhs=ident[:, :],
                                    is_transpose=True,
                                    start=(s == 0),
                                    stop=(s == 3),
                                    skip_group_check=True,
                                )
                            nc.vector.tensor_copy(
                                oT[(u, t)][:, :].rearrange(
                                    "p (a b) -> p a b", a=4
                                ),
                                pt[:, :, :],
                            )

                        fins.append((1, 0, emit_fin_a, t))
                        fins.append((3, 4 * 128, emit_fin_b, t))
                    else:
                        # ---- final phase: staggered per-q-sub normalization
                        # interleaved with the AV drain, then the Wo tail.
                        # Flush any queued fins (they produce oT(.,t3) that
                        # the tail's u0 matmuls read) and leftover wo units
                        # (their mm_pool allocations must all precede fpt's,
                        # or bufs=2 rotation would evict fpt mid-use).
                        while fins:
                            entry = fins.popleft()
                            entry[2]()
                            ledger["pe"] += entry[1] * PE_CY
                        while fillers:
                            entry = fillers.popleft()
                            entry[2]()
                            ledger["pe"] += entry[1] * PE_CY
                        fin_cells = {}
                        # one shared transpose bank for all 4 subs (pending-
                        # zero trick); allocated before any tail mm allocs so
                        # bufs=2 rotation never evicts it mid-use
                        fpt = mm_pool.tile(
                            [128, 4, 128], F16, tag="mm", name="fpt"
                        )

                        def fin_sub_a(s, avs=avs):
                            den = small_pool.tile(
                                [128, 2, 1], FP32, tag="den", name=f"fden{s}"
                            )
                            for i in range(2):
                                nc.scalar.copy(
                                    den[:, i, :], avs[i][:, s, DH : DH + 1]
                                )
                            rec = small_pool.tile(
                                [128, 2, 1], FP32, tag="rec32", name=f"frec{s}"
                            )
                            nc.vector.reciprocal_approx_fast(
                                rec[:, :, :], den[:, :, :]
                            )
                            ob2 = small_pool.tile(
                                [128, 1, 128], F16, tag="obo", name=f"fob{s}"
                            )
                            for i in range(2):
                                nc.vector.tensor_mul(
                                    ob2[:, 0, 64 * i : 64 * i + DH],
                                    avs[i][:, s, 0:DH],
                                    rec[:, i, :].broadcast_to((128, DH)),
                                )
                            fin_cells[s] = ob2

                        def fin_sub_b(s, u=u, t=t):
                            ob2 = fin_cells[s]
                            nc.tensor.matmul(
                                fpt[:, s, :],
                                lhsT=ob2[:, 0, :],
                                rhs=ident[:, :],
                                is_transpose=True,
                                start=(s == 0),
                                stop=(s == 3),
                                skip_group_check=True,
                            )
                            nc.vector.tensor_copy(
                                oT[(u, t)][:, 128 * s : 128 * (s + 1)],
                                fpt[:, s, :],
                            )

                        while pending_avs:
                            cc, exx, jj = pending_avs.popleft()
                            emit_av(cc, exx, jj)
                            s_stop = cc - 4 * t
                            if s_stop >= 0:
                                fin_sub_a(s_stop)
                                if s_stop >= 1:
                                    fin_sub_b(s_stop - 1)
                            pump_ledger(t + 1, extra=400)

                        # kernel tail: last tile's 8 Wo groups across 8
                        # borrowed PSUM slots. u=0 (full width, needs only
                        # oT(0,t3)) runs while the staggered fin chains drain;
                        # u=1 closes per q-sub as its oT slice lands; outputs
                        # leave as 4 per-sub DMAs split across SP and Pool
                        # queues so no issue serialization at the very end.
                        tl = N_ST - 1
                        tail_a = sc_pool.tile(
                            [128, 2 * S_TILE], FP32, tag="sc", name="tail_a"
                        )
                        tail_b = sc_pool.tile(
                            [128, 2 * S_TILE], FP32, tag="sc", name="tail_b"
                        )
                        slots = [
                            tail_a[:, 0:S_TILE],
                            tail_a[:, S_TILE:],
                            tail_b[:, 0:S_TILE],
                            tail_b[:, S_TILE:],
                            None,  # mm slots allocated after fin_sub_b(3)
                            None,
                            None,  # av slots borrowed once avs are dead
                            None,
                        ]

                        def wo_u0(dc):
                            nc.tensor.matmul(
                                slots[dc],
                                lhsT=wo_sb[:, 0, dc * 128 : (dc + 1) * 128],
                                rhs=oT[(0, tl)][:, :],
                                start=True,
                                stop=False,
                                skip_group_check=True,
                            )

                        def wo_u1(s, dc):
                            nc.tensor.matmul(
                                slots[dc][:, 128 * s : 128 * (s + 1)],
                                lhsT=wo_sb[:, 1, dc * 128 : (dc + 1) * 128],
                                rhs=oT[(1, tl)][:, 128 * s : 128 * (s + 1)],
                                start=False,
                                stop=(s == 3),
                                skip_group_check=True,
                            )

                        # outputs leave as two half-tile DMAs [128, 8, 256]
                        # (512B descriptors -- no small-elem latency penalty)
                        # on separate queues. Half-1 (subs 0-1, whose fins
                        # close early in the drain) is fully emitted per-dc
                        # right after u0 so its transfer overlaps the rest of
                        # the tail; half-2 closes per-dc behind it.
                        obh = {}
                        for h in range(2):
                            obh[h] = out_pool.tile(
                                [128, N_DC, 256], F16, tag="ob", name=f"obh{h}"
                            )

                        def half_copy(h, dc):
                            # dc 0-3 live in the two sc tiles as column pairs:
                            # one strided copy moves both dcs of a tile
                            if dc < 4:
                                if dc % 2 == 1:
                                    return
                                tile_ab = tail_a if dc == 0 else tail_b
                                src = tile_ab.rearrange("p (i m) -> p i m", i=2)[
                                    :, :, 256 * h : 256 * (h + 1)
                                ]
                                dsth = obh[h][:, dc : dc + 2, :]
                                if dc == 0:
                                    nc.scalar.copy(dsth, src)
                                else:
                                    nc.vector.tensor_copy(dsth, src)
                                return
                            src = slots[dc][:, 256 * h : 256 * (h + 1)]
                            dsth = obh[h][:, dc, :]
                            if dc % 2 == 0:
                                nc.scalar.copy(dsth, src)
                            else:
                                nc.vector.tensor_copy(dsth, src)

                        for dc in range(4):
                            wo_u0(dc)
                            wo_u1(0, dc)
                            wo_u1(1, dc)
                            if dc % 2 == 1:
                                half_copy(0, dc - 1)
                        fin_sub_b(3)  # before mm tail allocs (rotation safety)
                        slots[4] = mm_pool.tile(
                            [128, S_TILE], FP32, tag="mm", name="tail_m0"
                        )[:, :]
                        slots[5] = mm_pool.tile(
                            [128, S_TILE], FP32, tag="mm", name="tail_m1"
                        )[:, :]
                        slots[6] = av_pool.tile(
                            [128, S_TILE], FP32, tag="av", name="tail_v0"
                        )[:, :]
                        slots[7] = av_pool.tile(
                            [128, S_TILE], FP32, tag="av", name="tail_v1"
                        )[:, :]
                        for dc in range(4, 8):
                            wo_u0(dc)
                            wo_u1(0, dc)
                            wo_u1(1, dc)
                            half_copy(0, dc)
                        nc.sync.dma_start(
                            outT[:, tl * S_TILE : tl * S_TILE + 256].rearrange(
                                "(i p) m -> p i m", i=N_DC
                            ),
                            obh[0][:, :, :],
                        )
                        # subs 2-3 close dc-major and leave as two quarter
                        # pieces so the very last transfer is only 728ns
                        for dc in range(4):
                            wo_u1(2, dc)
                            wo_u1(3, dc)
                            if dc % 2 == 1:
                                half_copy(1, dc - 1)
                        nc.sync.dma_start(
                            outT[
                                0 : 4 * 128, tl * S_TILE + 256 : tl * S_TILE + 512
                            ].rearrange("(i p) m -> p i m", i=4),
                            obh[1][:, 0:4, :],
                        )
                        for dc in range(4, 8):
                            wo_u1(2, dc)
                            wo_u1(3, dc)
                            half_copy(1, dc)
                        nc.sync.dma_start(
                            outT[
                                4 * 128 :, tl * S_TILE + 256 : tl * S_TILE + 512
                            ].rearrange("(i p) m -> p i m", i=4),
                            obh[1][:, 4:8, :],
                        )
                # Wo for this tile becomes filler work for later attention
                # (the last tile's Wo is the kernel tail, emitted above)
                if t < N_ST - 1:
                    for dc in range(N_DC):
                        fillers.append(
                            (
                                99,
                                2 * S_TILE,
                                lambda t=t, dc=dc: emit_wo_group(t, dc),
                                t,
                            )
                        )
            while fillers:
                fillers.popleft()[2]()
            while fins:
                fins.popleft()[2]()
    nc.compile()
    return nc


_NC_CACHE = None


def _get_program():
    global _NC_CACHE
    if _NC_CACHE is None:
        _NC_CACHE = build_program()
    return _NC_CACHE


def _make_in_maps(x, Wq, Wk, Wv, Wo):
    f16 = np.float16
    xTs = [np.ascontiguousarray(x[b].T).astype(f16) for b in range(B)]
    in_maps = []
    for core in range(N_CORES):
        b, g = divmod(core, HG)
        r0, r1 = g * GM, (g + 1) * GM
        in_maps.append(
            {
                "xT": xTs[b],
                "wq": np.ascontiguousarray(Wq[r0:r1, :].T).astype(f16),
                "wk": np.ascontiguousarray(Wk[r0:r1, :].T).astype(f16),
                "wv": np.ascontiguousarray(Wv[r0:r1, :].T).astype(f16),
                "wo": np.ascontiguousarray(Wo[:, r0:r1].T).astype(f16),
            }
        )
    return in_maps


def kernel(x, Wq, Wk, Wv, Wo, **_unused):
    x = np.asarray(x, dtype=np.float32)
    Wq = np.asarray(Wq, dtype=np.float32)
    Wk = np.asarray(Wk, dtype=np.float32)
    Wv = np.asarray(Wv, dtype=np.float32)
    Wo = np.asarray(Wo, dtype=np.float32)

    nc = _get_program()
    in_maps = _make_in_maps(x, Wq, Wk, Wv, Wo)
    res = run_bass_kernel_spmd(nc, in_maps, core_ids=list(range(N_CORES)))
    out = np.zeros((B, S, D), dtype=np.float64)
    for core in range(N_CORES):
        b = core // HG
        out[b] += res.results[core]["outT"].T.astype(np.float64)
    return out.astype(np.float32)


# revision 35
# speedup vs baseline: 1.1213x; 1.0427x over previous
"""Causal multi-head attention (B=2, S=2048, D=1024, H=16) on 8 TRN2 NeuronCores.

Sharding: batch*heads across cores. Core c handles batch c//4 and the 4 heads
g*4..g*4+3 where g = c%4. Weights are sliced per core (Megatron-style column
split of Wq/Wk/Wv, row split of Wo); each core produces a partial projected
output [D, S] (transposed) and the host sums the 4 partials per batch.

Everything on-chip is kept transposed ([feature, seq]) so no transposes are
ever needed on the forward path:
  qT/kT = wq/wk^T @ xT            (PE, fp16, contraction over D)
  v     = x @ Wv^T                (fp16; s on partitions, + ones col appended)
  sT    = k @ qT  [s_k=128, s_q]  (PE, contraction over dh=64, 2 heads packed
                                   via base-partition 0/64 row groups)
  eT    = exp(sT/8) -> fp16, then an on-chip 0/1 mask (built once with
          affine_select) zeroes the 128-wide causal band (GPSIMD)
  avB   = eT-subtile^T @ v_aug    [q=128, 65]  per (head, q-sub): q on
          PARTITIONS, so the matmul free size is 65, not 512 -- half the PE
          cycles of the [65, 512] orientation, and the softmax denominator
          (col 64) comes along free.
  o     = avB[:, 0:64] * recip(avB[:, 64]) broadcast along the FREE dim (DVE)
          -- no denominator-broadcast matmul needed at all.
  oT    = PE-transpose(o, identity)  [dims 128, q 128] per q-sub (128 cyc)
  partialT = wo^T-chunks @ oT     (PE, fp16, contraction over 256 head dims)

All 16-bit tensors are fp16 (not bf16): same PE/DMA cost, 4 more mantissa
bits. Partials stream out as fp16 (host sums in float64).

Scheduling: engines execute their instruction streams in emission order, so
the kernel is emitted as ONE interleaved stream. Attention chunks (latency-
bound on the PE->ACT->GPSIMD->PE chain) are diluted with fine-grained filler
units (2-4 matmuls each): QKV work for later tiles, deferred Wo work for
earlier tiles, and deferred per-phase normalization units (fins: fin_a =
den/recip/divide on ACT+DVE, fin_b = PE transposes + oT copy, popped 1 and 3
chunks into the next phase respectively so the PE never parks on the DVE
chain). A per-phase emission-time ledger of ACT-vs-PE nanoseconds pumps
fillers exactly when the scalar engine (exp) would fall behind; AV matmuls
trail their scores by AV_LAG chunks. The chunk-major prologue advances all
four k/q accumulators per arriving x-block pair. The final phase staggers
normalization per q-sub (q-sub s's AV accumulation closes at diag chunk
12+s), interleaving per-sub fins with the AV drain, then runs Wo u=0 across 8
borrowed PSUM slots, per-sub u=1 closes, and 4 per-sub output DMAs split
across the SP (HWDGE) and Pool (SWDGE) queues.
"""

from collections import deque

import numpy as np

import concourse.bass as bass
import concourse.mybir as mybir
import concourse.tile as tile
from concourse import bacc, masks
from concourse.bass_utils import run_bass_kernel_spmd

B = 2
S = 2048
D = 1024
H = 16
DH = 64
N_CORES = 8
HG = H // 4  # 4 heads per core
GM = 4 * DH  # 256 head dims per core
FP32 = mybir.dt.float32
F16 = mybir.dt.float16
F8 = mybir.dt.float8e4
DR = mybir.MatmulPerfMode.DoubleRow
WSC = 64.0  # host-side weight scale (undone via exp scale and wo)

S_TILE = 512  # q-tile width (PSUM bank)
N_ST = S // S_TILE  # 4
KC = 128  # k-chunk (partition dim of scoresT)
N_KC = S // KC  # 16
N_DC = D // 128  # 8 d-chunks
AV_LAG = 4  # chunks between scores and their AV matmuls (hides exp+mask latency)


def build_program():
    nc = bacc.Bacc("TRN2", target_bir_lowering=False, debug=False)

    # x and the QKV weights arrive as fp8 hi/lo residual pairs (host-split;
    # weights pre-scaled by WSC so the lo residual clears the fp8 subnormal
    # floor, and pre-arranged to the [128, c, m] SBUF layout). The DoubleRow
    # matmuls then run at 0.5 cycles/row -- QKV costs 0.75x of fp16 via the
    # 3-term scheme hi*hi + lo*hi + hi*lo.
    xh = nc.dram_tensor("xh", [D, S], F8, kind="ExternalInput")
    xl = nc.dram_tensor("xl", [D, S], F8, kind="ExternalInput")
    wsplit = {}
    for name in ("q", "k", "v"):
        for kind in ("h", "l"):
            wsplit[(name, kind)] = nc.dram_tensor(
                f"w{name}{kind}", [128, N_DC * GM], F8, kind="ExternalInput"
            )
    wo = nc.dram_tensor("wo", [GM, D], F16, kind="ExternalInput")
    outT = nc.dram_tensor("outT", [D, S], F16, kind="ExternalOutput")

    with tile.TileContext(nc) as tc:
        with (
            tc.tile_pool(name="persist", bufs=1) as persist,
            tc.tile_pool(name="xb", bufs=8) as xb_pool,
            tc.tile_pool(name="exp", bufs=10) as exp_pool,
            tc.tile_pool(name="small", bufs=4) as small_pool,
            tc.tile_pool(name="outsb", bufs=4) as out_pool,
            tc.tile_pool(name="mm", bufs=2, space="PSUM") as mm_pool,
            tc.tile_pool(name="scores", bufs=2, space="PSUM") as sc_pool,
            tc.tile_pool(name="av", bufs=2, space="PSUM") as av_pool,
        ):
            # ---- persistent SBUF tensors ----
            wo_sb = persist.tile([128, 2, D], F16, tag="wo")
            ones_col = persist.tile([128, 1], FP32, tag="ones")
            ident = persist.tile([128, 128], F16, tag="ident")
            w_sb = {}
            for name in ("q", "k", "v"):
                for kind in ("h", "l"):
                    w_sb[(name, kind)] = persist.tile(
                        [128, N_DC, GM], F8, tag=f"w{name}{kind}",
                        name=f"w{name}{kind}sb"
                    )
            # wk rides the Pool SWDGE queue, issued in parallel with the SP
            # (HWDGE) x stream
            for kind in ("h", "l"):
                nc.gpsimd.dma_start(
                    w_sb[("k", kind)][:, :, :],
                    wsplit[("k", kind)].rearrange("p (c m) -> p c m", m=GM),
                )
            nc.vector.memset(ones_col[:, :], 1.0)
            masks.make_identity(nc, ident[:, :])
            # causal mask patterns, generated on-chip (no DMA):
            # mask4[p, j, q] = 1.0 iff 128*j + p <= q  (memset on DVE: Pool's
            # startup time is the first-weight-DMA critical path)
            mask4 = persist.tile([128, 4, S_TILE], FP32, tag="mask4")
            nc.vector.memset(mask4[:, :, :], 1.0)
            for j in range(4):
                nc.gpsimd.affine_select(
                    mask4[:, j, :],
                    mask4[:, j, :],
                    pattern=[[1, S_TILE]],
                    compare_op=mybir.AluOpType.is_ge,
                    fill=0.0,
                    base=-128 * j,
                    channel_multiplier=-1,
                )

            qT = {}  # (u, t) -> [128, 512]   2 heads stacked (rows 0-63 / 64-127)
            kT = {}
            vt = {}  # c16 -> [128, HG, 65]   v chunk with ones col per head
            oT = {}  # (u, t) -> [128, 512]   attention out, dims on partitions
            for t in range(N_ST):
                for u in range(2):
                    qT[(u, t)] = persist.tile(
                        [128, S_TILE], F16, tag=f"qT{u}{t}", name=f"qT{u}{t}"
                    )
                    kT[(u, t)] = persist.tile(
                        [128, S_TILE], F16, tag=f"kT{u}{t}", name=f"kT{u}{t}"
                    )
                    oT[(u, t)] = persist.tile(
                        [128, S_TILE], F16, tag=f"oT{u}{t}", name=f"oT{u}{t}"
                    )
            for c16 in range(N_KC):
                vt[c16] = persist.tile(
                    [128, HG, DH + 1], F16, tag=f"v{c16}", name=f"v{c16}"
                )

            xb = {}  # (kind, even c0, t) -> [128, 2, S_TILE] DoubleRow pair

            def load_xb(t, h, eng=None):
                # one DMA per (hi/lo, 4-chunk half-tile): [128, 4, 512] fp8
                for kind, src_t in (("h", xh), ("l", xl)):
                    blk = xb_pool.tile(
                        [128, 4, S_TILE], F8, tag="xb", name=f"xb{kind}{h}_{t}"
                    )
                    (eng or nc.sync).dma_start(
                        blk[:, :, :],
                        src_t[
                            h * 512 : (h + 1) * 512,
                            t * S_TILE : (t + 1) * S_TILE,
                        ].rearrange("(i p) m -> p i m", i=4),
                        )
                    for pr in range(2):
                        xb[(kind, 4 * h + 2 * pr, t)] = blk[
                            :, 2 * pr : 2 * pr + 2, :
                        ]

            # ---- DMA stream order ----
            # tile-0 inputs first (wk on the Pool queue above, x + wq on SP),
            # then x tiles 1..3 stream ahead of their QKV filler units, then
            # wo (needed late).
            def load_w(name, kind, eng=None):
                (eng or nc.sync).dma_start(
                    w_sb[(name, kind)][:, :, :],
                    wsplit[(name, kind)].rearrange("p (c m) -> p c m", m=GM),
                )

            load_xb(0, 0)
            load_w("q", "h")
            load_w("q", "l")
            load_xb(0, 1)
            load_w("v", "h", eng=nc.gpsimd)
            load_w("v", "l", eng=nc.gpsimd)
            for h in range(2):
                load_xb(1, h, eng=nc.gpsimd)
            nc.sync.dma_start(wo_sb[:, :, :], wo.rearrange("(u p) d -> p u d", p=128))
            for t in range(2, N_ST):
                for h in range(2):
                    load_xb(t, h)

            # ---- emission thunks ----
            # Group PSUM tiles are created lazily by the first unit of each
            # group (cell dict) so mm_pool's buffer rotation follows actual
            # use order, not enqueue order.
            def qk_matmuls(name, u, t, cell, pr):
                # one DoubleRow pair (chunks 2pr, 2pr+1), 3 residual terms
                if pr == 0:
                    cell["ps"] = mm_pool.tile(
                        [128, S_TILE], FP32, tag="mm", name=f"ps{name}{u}{t}"
                    )
                ps = cell["ps"]
                c0 = 2 * pr
                lh = w_sb[(name, "h")][:, c0 : c0 + 2, u * 128 : (u + 1) * 128]
                ll = w_sb[(name, "l")][:, c0 : c0 + 2, u * 128 : (u + 1) * 128]
                rh = xb[("h", c0, t)]
                rl = xb[("l", c0, t)]
                for i, (lw, rx) in enumerate(((lh, rh), (ll, rh), (lh, rl))):
                    nc.tensor.matmul(
                        ps[:, :],
                        lhsT=lw,
                        rhs=rx,
                        start=(pr == 0 and i == 0),
                        stop=(pr == 3 and i == 2),
                        perf_mode=DR,
                        skip_group_check=True,
                    )
                if pr == 3:
                    dst = kT if name == "k" else qT
                    nc.vector.tensor_copy(dst[(u, t)][:, :], ps[:, :])

            def v_matmuls(t, s4, cell, pr0, pr1):
                # DoubleRow pairs pr0..pr1-1, x stationary, 3 residual terms
                c16 = 4 * t + s4
                if pr0 == 0:
                    cell["ps"] = mm_pool.tile(
                        [128, GM], FP32, tag="mm", name=f"psv{c16}"
                    )
                ps = cell["ps"]
                for pr in range(pr0, pr1):
                    c0 = 2 * pr
                    s_sl = slice(s4 * 128, (s4 + 1) * 128)
                    lh = xb[("h", c0, t)][:, :, s_sl]
                    ll = xb[("l", c0, t)][:, :, s_sl]
                    rh = w_sb[("v", "h")][:, c0 : c0 + 2, :]
                    rl = w_sb[("v", "l")][:, c0 : c0 + 2, :]
                    for i, (lx, rw) in enumerate(((lh, rh), (ll, rh), (lh, rl))):
                        nc.tensor.matmul(
                            ps[:, :],
                            lhsT=lx,
                            rhs=rw,
                            start=(pr == 0 and i == 0),
                            stop=(pr == 3 and i == 2),
                            perf_mode=DR,
                            skip_group_check=True,
                        )
                if pr1 == 4:
                    nc.vector.tensor_copy(
                        vt[c16][:, :, 0:DH], ps.rearrange("p (h d) -> p h d", h=HG)
                    )
                    nc.gpsimd.tensor_copy(
                        vt[c16][:, :, DH : DH + 1],
                        ones_col[:, 0:1].broadcast_to((128, HG, 1)),
                    )

            def emit_wo_group(t, dc, on_act=False):
                po = mm_pool.tile([128, S_TILE], FP32, tag="mm", name=f"po{t}{dc}")
                for u in range(2):
                    nc.tensor.matmul(
                        po[:, :],
                        lhsT=wo_sb[:, u, dc * 128 : (dc + 1) * 128],
                        rhs=oT[(u, t)][:, :],
                        start=(u == 0),
                        stop=(u == 1),
                    )
                ob = out_pool.tile([128, S_TILE], F16, tag="ob")
                if on_act:  # tail: split the copy across ACT+DVE so the PSUM
                    # bank frees at PE pace, not copy pace
                    nc.scalar.copy(ob[:, 0 : S_TILE // 2], po[:, 0 : S_TILE // 2])
                    nc.vector.tensor_copy(ob[:, S_TILE // 2 :], po[:, S_TILE // 2 :])
                else:
                    nc.vector.tensor_copy(ob[:, :], po[:, :])
                nc.sync.dma_start(
                    outT[dc * 128 : (dc + 1) * 128, t * S_TILE : (t + 1) * S_TILE],
                    ob[:, :],
                )

            # filler queue: (tile_tag, cost_cycles, thunk). Attention chunks pump
            # filler units between chunks to keep the PE dense while exp/mask
            # latency elapses. QKV units are tagged with their tile (flushed
            # before that tile's attention); Wo units are tagged 99 (pump/drain
            # only -- they are enqueued once their oT inputs exist).
            fillers = deque()

            def enqueue_qkv_units(t):
                for name in ("k", "q"):
                    for u in range(2):
                        cell = {}
                        for pr in range(4):
                            fillers.append(
                                (
                                    t,
                                    3 * S_TILE // 2,
                                    lambda name=name, u=u, t=t, cell=cell, pr=pr: qk_matmuls(
                                        name, u, t, cell, pr
                                    ),
                                )
                            )
                # v(t) is only consumed by (t,0)'s AV drain, so its units are
                # tagged t+0.5: they skip the phase-start flush and instead
                # pump as filler DURING (t,0), where late phases are starved
                for s4 in range(4):
                    cell = {}
                    for pr0 in range(0, 4, 2):
                        fillers.append(
                            (
                                t + 0.5,
                                3 * GM,
                                lambda t=t, s4=s4, cell=cell, pr0=pr0: v_matmuls(
                                    t, s4, cell, pr0, pr0 + 2
                                ),
                            )
                        )

            # --- emission-time ACT-vs-PE ledger (reset per attention phase).
            # exp work accumulates act_ns; attention matmuls + pumped fillers
            # accumulate pe_ns. Pump fillers whenever ACT is ahead, so the PE
            # stream is diluted exactly where the scalar engine needs time.
            PE_CY = 1.0 / 2.4  # ns per cycle at peak
            SLACK = 900.0  # exp-pipeline fill depth: don't pump before ACT is
            # genuinely ahead of the PE stream in wall-clock terms
            ledger = {"pe": 0.0, "act": -SLACK}

            # deferred normalization units: (min_chunk, cost, thunk) popped by
            # pumps min_chunk chunks into the NEXT phase, when their upstream
            # ACT/DVE chain has drained
            fins = deque()
            phase_chunk = {"c": 0}

            def pump_ledger(max_tag, extra=0.0):
                ledger["act"] += extra
                while ledger["pe"] < ledger["act"] and (fins or fillers):
                    if fins and phase_chunk["c"] >= fins[0][0]:
                        entry = fins.popleft()
                        cost, thunk = entry[1], entry[2]
                    elif fillers:
                        entry = fillers[0]
                        tag, cost, thunk = entry[0], entry[1], entry[2]
                        if tag == 99:
                            wo_tile = entry[3]
                            if any(f[3] == wo_tile for f in fins):
                                # a wo(t) unit must never be emitted while a
                                # fin producing its oT input is still queued:
                                # the PE stream is in-order, so that would
                                # deadlock
                                return
                            if max_tag < N_ST:
                                # hold wo units for the filler-starved t3
                                # phases
                                return
                        elif tag > max_tag:
                            return
                        fillers.popleft()
                    else:
                        return
                    thunk()
                    ledger["pe"] += cost * PE_CY

            def flush_tile(t):
                # 99 = wo units (pump/drain only, reserved as late filler)
                while fillers and fillers[0][0] <= t:
                    _, _, thunk = fillers.popleft()
                    thunk()

            # prologue, chunk-major: all four k/q accumulation groups advance
            # per arriving x block (their PSUM lives in sc_pool, idle during
            # the prologue), so the PE has ~850ns of work per ~1.5us DMA chunk
            # instead of ~430ns. v groups become tile-0 fillers.
            sc0 = sc_pool.tile([128, 2 * S_TILE], FP32, tag="sc", name="pro_sc0")
            sc1 = sc_pool.tile([128, 2 * S_TILE], FP32, tag="sc", name="pro_sc1")
            pro = {("k", 0): sc0[:, 0:S_TILE], ("q", 0): sc0[:, S_TILE:],
                   ("k", 1): sc1[:, 0:S_TILE], ("q", 1): sc1[:, S_TILE:]}
            for s4 in range(4):
                cell = {}
                for pr0 in range(0, 4, 2):
                    fillers.append(
                        (
                            0.5,
                            3 * GM,
                            lambda s4=s4, cell=cell, pr0=pr0: v_matmuls(
                                0, s4, cell, pr0, pr0 + 2
                            ),
                        )
                    )
            for pr in range(4):
                c0 = 2 * pr
                for name in ("k", "q"):
                    for u in range(2):
                        lh = w_sb[(name, "h")][
                            :, c0 : c0 + 2, u * 128 : (u + 1) * 128
                        ]
                        ll = w_sb[(name, "l")][
                            :, c0 : c0 + 2, u * 128 : (u + 1) * 128
                        ]
                        rh = xb[("h", c0, 0)]
                        rl = xb[("l", c0, 0)]
                        for i, (lw, rx) in enumerate(
                            ((lh, rh), (ll, rh), (lh, rl))
                        ):
                            nc.tensor.matmul(
                                pro[(name, u)],
                                lhsT=lw,
                                rhs=rx,
                                start=(pr == 0 and i == 0),
                                stop=(pr == 3 and i == 2),
                                perf_mode=DR,
                                skip_group_check=True,
                            )
            for u in range(2):  # ACT is idle during the prologue; keep DVE free
                nc.scalar.copy(kT[(u, 0)][:, :], pro[("k", u)])
                nc.scalar.copy(qT[(u, 0)][:, :], pro[("q", u)])
            for t in range(1, N_ST):
                enqueue_qkv_units(t)

            for t in range(N_ST):
                nch = 4 * t + 4
                for hp in range(2):
                    u = hp
                    final = t == N_ST - 1 and hp == 1
                    if t > 0 or hp > 0:
                        # (t,0) needs kq u=0; (t,1) additionally needs u=1
                        flush_tile(t if hp == 0 else t + 0.3)
                    ledger["pe"] = 0.0
                    ledger["act"] = -SLACK
                    phase_chunk["c"] = 0
                    avs = [
                        av_pool.tile(
                            [128, 4, DH + 1], FP32, tag="av", name=f"av{t}{hp}{i}"
                        )
                        for i in range(2)
                    ]
                    pending_avs = deque()  # AV trails scores by AV_LAG chunks

                    def emit_av(cc, exx, jj, avs=avs, hp=hp, t=t, nch=nch):
                        # q-on-partitions AV: per (head, q-sub 128) the matmul
                        # free size is just 65 (64 dims + the ones/denominator
                        # col); q-sub s's accumulation closes at diag chunk
                        # 4t+s. PSUM accumulation-group starts are ZERO-REGION
                        # (2KB bank) granular: exactly ONE start and ONE stop
                        # per av tile -- the bank-wide pending-zero from the
                        # single start zeroes each sub-region on first touch.
                        s0 = max(jj, 0)
                        for i in range(2):
                            for s in range(s0, 4):
                                nc.tensor.matmul(
                                    avs[i][:, s, :],
                                    lhsT=exx[
                                        :,
                                        i * S_TILE + 128 * s : i * S_TILE
                                        + 128 * (s + 1),
                                    ],
                                    rhs=vt[cc][:, 2 * hp + i, :],
                                    start=(cc == 0 and s == s0),
                                    stop=(cc == nch - 1 and s == 3),
                                    skip_group_check=True,
                                )
                        ledger["pe"] += 2 * (4 - s0) * (DH + 1) * PE_CY

                    for c in range(nch):
                        # Diagonal chunks only touch q columns >= 128j
                        # (causal): scores / exp / AV skip the masked prefix.
                        j = c - 4 * t
                        q0 = 128 * j if j >= 0 else 0
                        w = S_TILE - q0
                        sc = sc_pool.tile([128, 2 * S_TILE], FP32, tag="sc")
                        for i in range(2):  # head parity: rows 0-63 / 64-127
                            bp = 64 * i
                            nc.tensor.matmul(
                                sc[:, i * S_TILE + q0 : (i + 1) * S_TILE],
                                lhsT=kT[(u, c // 4)][
                                    bp : bp + DH, (c % 4) * 128 : (c % 4 + 1) * 128
                                ],
                                rhs=qT[(u, t)][bp : bp + DH, q0:],
                                start=True,
                                stop=True,
                            )
                        ledger["pe"] += 2 * w * PE_CY
                        ex = exp_pool.tile([128, 2 * S_TILE], F16, tag="ex")
                        exv = ex.rearrange("p (i n) -> p i n", i=2)[:, :, q0:]
                        scv = sc.rearrange("p (i n) -> p i n", i=2)[:, :, q0:]
                        nc.scalar.activation(
                            exv, scv, mybir.ActivationFunctionType.Exp, scale=0.125 / (WSC * WSC)
                        )
                        ledger["act"] += 2 * w * 0.833 + 242
                        if j >= 0:
                            # zero the causal triangle (mask is 0/1, exact).
                            # Only the band [q0, 128j+128) needs masking; one
                            # op covers both head slots via a broadcast mask.
                            bhi = 128 * j + 128
                            exb = ex.rearrange("p (i n) -> p i n", i=2)[
                                :, :, q0:bhi
                            ]
                            nc.gpsimd.tensor_mul(
                                exb,
                                exb,
                                mask4[:, j : j + 1, q0:bhi].broadcast_to(
                                    (128, 2, bhi - q0)
                                ),
                            )

                        pending_avs.append((c, ex, j))
                        phase_chunk["c"] = c
                        pump_ledger(t + 1)
                        if len(pending_avs) > AV_LAG:
                            emit_av(*pending_avs.popleft())
                    if hp == 0:
                        flush_tile(t + 0.5)  # v(t) needed by the AV drain

                    if not final:
                        while pending_avs:
                            emit_av(*pending_avs.popleft())
                            # the exp tail is still draining on ACT in
                            # wall-clock terms; keep filler between the drain
                            pump_ledger(t + 1, extra=400)
                        # normalization: rec = 1/den per (head, q-sub) on DVE,
                        # o = av * rec via FREE-dim broadcast (q is on
                        # partitions), then 4 PE transposes into oT. Split so
                        # the PE piece (fin_b) pops 3 chunks into the next
                        # phase, when fin_a's DVE chain has drained.
                        cell = {}

                        def emit_fin_a(avs=avs, cell=cell):
                            den = small_pool.tile([128, 2, 4], FP32, tag="den")
                            for i in range(2):
                                nc.vector.tensor_copy(den[:, i, :], avs[i][:, :, DH])
                            rec = small_pool.tile([128, 2, 4], FP32, tag="rec32")
                            nc.vector.reciprocal_approx_fast(
                                rec[:, :, :], den[:, :, :]
                            )
                            ob2 = small_pool.tile([128, 4, 128], F16, tag="obo")
                            for i in range(2):
                                nc.vector.tensor_mul(
                                    ob2[:, :, 64 * i : 64 * i + DH],
                                    avs[i][:, :, 0:DH],
                                    rec[:, i, :]
                                    .rearrange("p (a b) -> p a b", b=1)
                                    .broadcast_to((128, 4, DH)),
                                )
                            cell["ob2"] = ob2

                        def emit_fin_b(cell=cell, u=u, t=t):
                            # all 4 transposes share ONE PSUM bank: a single
                            # group start (pending-zero covers the bank, each
                            # sub-region zeroes on first touch) avoids 4x
                            # mm_pool rotation and 4 small copies
                            ob2 = cell["ob2"]
                            pt = mm_pool.tile(
                                [128, 4, 128], F16, tag="mm", name=f"pt{t}{u}"
                            )
                            for s in range(4):
                                nc.tensor.matmul(
                                    pt[:, s, :],
                                    lhsT=ob2[:, s, :],
                                    rhs=ident[:, :],
                                    is_transpose=True,
                                    start=(s == 0),
                                    stop=(s == 3),
                                    skip_group_check=True,
                                )
                            nc.vector.tensor_copy(
                                oT[(u, t)][:, :].rearrange(
                                    "p (a b) -> p a b", a=4
                                ),
                                pt[:, :, :],
                            )

                        fins.append((1, 0, emit_fin_a, t))
                        fins.append((3, 4 * 128, emit_fin_b, t))
                    else:
                        # ---- final phase: staggered per-q-sub normalization
                        # interleaved with the AV drain, then the Wo tail.
                        # Flush any queued fins (they produce oT(.,t3) that
                        # the tail's u0 matmuls read) and leftover wo units
                        # (their mm_pool allocations must all precede fpt's,
                        # or bufs=2 rotation would evict fpt mid-use).
                        while fins:
                            entry = fins.popleft()
                            entry[2]()
                            ledger["pe"] += entry[1] * PE_CY
                        while fillers:
                            entry = fillers.popleft()
                            entry[2]()
                            ledger["pe"] += entry[1] * PE_CY
                        fin_cells = {}
                        # one shared transpose bank for all 4 subs (pending-
                        # zero trick); allocated before any tail mm allocs so
                        # bufs=2 rotation never evicts it mid-use
                        fpt = mm_pool.tile(
                            [128, 4, 128], F16, tag="mm", name="fpt"
                        )

                        def fin_sub_a(s, avs=avs):
                            den = small_pool.tile(
                                [128, 2, 1], FP32, tag="den", name=f"fden{s}"
                            )
                            for i in range(2):
                                nc.scalar.copy(
                                    den[:, i, :], avs[i][:, s, DH : DH + 1]
                                )
                            rec = small_pool.tile(
                                [128, 2, 1], FP32, tag="rec32", name=f"frec{s}"
                            )
                            nc.vector.reciprocal_approx_fast(
                                rec[:, :, :], den[:, :, :]
                            )
                            ob2 = small_pool.tile(
                                [128, 1, 128], F16, tag="obo", name=f"fob{s}"
                            )
                            for i in range(2):
                                nc.vector.tensor_mul(
                                    ob2[:, 0, 64 * i : 64 * i + DH],
                                    avs[i][:, s, 0:DH],
                                    rec[:, i, :].broadcast_to((128, DH)),
                                )
                            fin_cells[s] = ob2

                        def fin_sub_b(s, u=u, t=t):
                            ob2 = fin_cells[s]
                            nc.tensor.matmul(
                                fpt[:, s, :],
                                lhsT=ob2[:, 0, :],
                                rhs=ident[:, :],
                                is_transpose=True,
                                start=(s == 0),
                                stop=(s == 3),
                                skip_group_check=True,
                            )
                            nc.vector.tensor_copy(
                                oT[(u, t)][:, 128 * s : 128 * (s + 1)],
                                fpt[:, s, :],
                            )

                        while pending_avs:
                            cc, exx, jj = pending_avs.popleft()
                            emit_av(cc, exx, jj)
                            s_stop = cc - 4 * t
                            if s_stop >= 0:
                                fin_sub_a(s_stop)
                                if s_stop >= 1:
                                    fin_sub_b(s_stop - 1)
                            pump_ledger(t + 1, extra=400)

                        # kernel tail: last tile's 8 Wo groups across 8
                        # borrowed PSUM slots. u=0 (full width, needs only
                        # oT(0,t3)) runs while the staggered fin chains drain;
                        # u=1 closes per q-sub as its oT slice lands; outputs
                        # leave as 4 per-sub DMAs split across SP and Pool
                        # queues so no issue serialization at the very end.
                        tl = N_ST - 1
                        tail_a = sc_pool.tile(
                            [128, 2 * S_TILE], FP32, tag="sc", name="tail_a"
                        )
                        tail_b = sc_pool.tile(
                            [128, 2 * S_TILE], FP32, tag="sc", name="tail_b"
                        )
                        slots = [
                            tail_a[:, 0:S_TILE],
                            tail_a[:, S_TILE:],
                            tail_b[:, 0:S_TILE],
                            tail_b[:, S_TILE:],
                            None,  # mm slots allocated after fin_sub_b(3)
                            None,
                            None,  # av slots borrowed once avs are dead
                            None,
                        ]

                        def wo_u0(dc):
                            nc.tensor.matmul(
                                slots[dc],
                                lhsT=wo_sb[:, 0, dc * 128 : (dc + 1) * 128],
                                rhs=oT[(0, tl)][:, :],
                                start=True,
                                stop=False,
                                skip_group_check=True,
                            )

                        def wo_u1(s, dc):
                            nc.tensor.matmul(
                                slots[dc][:, 128 * s : 128 * (s + 1)],
                                lhsT=wo_sb[:, 1, dc * 128 : (dc + 1) * 128],
                                rhs=oT[(1, tl)][:, 128 * s : 128 * (s + 1)],
                                start=False,
                                stop=(s == 3),
                                skip_group_check=True,
                            )

                        # outputs leave as two half-tile DMAs [128, 8, 256]
                        # (512B descriptors -- no small-elem latency penalty)
                        # on separate queues. Half-1 (subs 0-1, whose fins
                        # close early in the drain) is fully emitted per-dc
                        # right after u0 so its transfer overlaps the rest of
                        # the tail; half-2 closes per-dc behind it.
                        obh = {}
                        for h in range(2):
                            obh[h] = out_pool.tile(
                                [128, N_DC, 256], F16, tag="ob", name=f"obh{h}"
                            )

                        def half_copy(h, dc):
                            # dc 0-3 live in the two sc tiles as column pairs:
                            # one strided copy moves both dcs of a tile
                            if dc < 4:
                                if dc % 2 == 1:
                                    return
                                tile_ab = tail_a if dc == 0 else tail_b
                                src = tile_ab.rearrange("p (i m) -> p i m", i=2)[
                                    :, :, 256 * h : 256 * (h + 1)
                                ]
                                dsth = obh[h][:, dc : dc + 2, :]
                                if dc == 0:
                                    nc.scalar.copy(dsth, src)
                                else:
                                    nc.vector.tensor_copy(dsth, src)
                                return
                            src = slots[dc][:, 256 * h : 256 * (h + 1)]
                            dsth = obh[h][:, dc, :]
                            if dc % 2 == 0:
                                nc.scalar.copy(dsth, src)
                            else:
                                nc.vector.tensor_copy(dsth, src)

                        for dc in range(4):
                            wo_u0(dc)
                            wo_u1(0, dc)
                            wo_u1(1, dc)
                            if dc % 2 == 1:
                                half_copy(0, dc - 1)
                        fin_sub_b(3)  # before mm tail allocs (rotation safety)
                        slots[4] = mm_pool.tile(
                            [128, S_TILE], FP32, tag="mm", name="tail_m0"
                        )[:, :]
                        slots[5] = mm_pool.tile(
                            [128, S_TILE], FP32, tag="mm", name="tail_m1"
                        )[:, :]
                        slots[6] = av_pool.tile(
                            [128, S_TILE], FP32, tag="av", name="tail_v0"
                        )[:, :]
                        slots[7] = av_pool.tile(
                            [128, S_TILE], FP32, tag="av", name="tail_v1"
                        )[:, :]
                        for dc in range(4, 8):
                            wo_u0(dc)
                            wo_u1(0, dc)
                            wo_u1(1, dc)
                            half_copy(0, dc)
                        nc.sync.dma_start(
                            outT[:, tl * S_TILE : tl * S_TILE + 256].rearrange(
                                "(i p) m -> p i m", i=N_DC
                            ),
                            obh[0][:, :, :],
                        )
                        # subs 2-3 close dc-major and leave as two quarter
                        # pieces so the very last transfer is only 728ns
                        for dc in range(4):
                            wo_u1(2, dc)
                            wo_u1(3, dc)
                            if dc % 2 == 1:
                                half_copy(1, dc - 1)
                        nc.sync.dma_start(
                            outT[
                                0 : 4 * 128, tl * S_TILE + 256 : tl * S_TILE + 512
                            ].rearrange("(i p) m -> p i m", i=4),
                            obh[1][:, 0:4, :],
                        )
                        for dc in range(4, 8):
                            wo_u1(2, dc)
                            wo_u1(3, dc)
                            half_copy(1, dc)
                        nc.sync.dma_start(
                            outT[
                                4 * 128 :, tl * S_TILE + 256 : tl * S_TILE + 512
                            ].rearrange("(i p) m -> p i m", i=4),
                            obh[1][:, 4:8, :],
                        )
                # Wo for this tile becomes filler work for later attention
                # (the last tile's Wo is the kernel tail, emitted above)
                if t < N_ST - 1:
                    for dc in range(N_DC):
                        fillers.append(
                            (
                                99,
                                2 * S_TILE,
                                lambda t=t, dc=dc: emit_wo_group(t, dc),
                                t,
                            )
                        )
            while fillers:
                fillers.popleft()[2]()
            while fins:
                fins.popleft()[2]()
    nc.compile()
    return nc


_NC_CACHE = None


def _get_program():
    global _NC_CACHE
    if _NC_CACHE is None:
        _NC_CACHE = build_program()
    return _NC_CACHE


def _split_f8(a):
    import ml_dtypes

    f8 = ml_dtypes.float8_e4m3
    hi = a.astype(f8)
    lo = (a - hi.astype(np.float32)).astype(f8)
    return np.ascontiguousarray(hi), np.ascontiguousarray(lo)


def _w_layout(w):
    # [D, GM] -> [128, N_DC * GM] matching the SBUF [p, c, m] tile layout
    return np.ascontiguousarray(
        w.reshape(N_DC, 128, GM).transpose(1, 0, 2).reshape(128, N_DC * GM)
    )


def _make_in_maps(x, Wq, Wk, Wv, Wo):
    f16 = np.float16
    xs = [_split_f8(np.ascontiguousarray(x[b].T)) for b in range(B)]
    in_maps = []
    for core in range(N_CORES):
        b, g = divmod(core, HG)
        r0, r1 = g * GM, (g + 1) * GM
        m = {"xh": xs[b][0], "xl": xs[b][1]}
        for name, W in (("q", Wq), ("k", Wk), ("v", Wv)):
            hi, lo = _split_f8(_w_layout(
                np.ascontiguousarray(W[r0:r1, :].T) * np.float32(WSC)
            ))
            m[f"w{name}h"] = hi
            m[f"w{name}l"] = lo
        m["wo"] = np.ascontiguousarray(Wo[:, r0:r1].T / np.float32(WSC)).astype(f16)
        in_maps.append(m)
    return in_maps


def kernel(x, Wq, Wk, Wv, Wo, **_unused):
    x = np.asarray(x, dtype=np.float32)
    Wq = np.asarray(Wq, dtype=np.float32)
    Wk = np.asarray(Wk, dtype=np.float32)
    Wv = np.asarray(Wv, dtype=np.float32)
    Wo = np.asarray(Wo, dtype=np.float32)

    nc = _get_program()
    in_maps = _make_in_maps(x, Wq, Wk, Wv, Wo)
    res = run_bass_kernel_spmd(nc, in_maps, core_ids=list(range(N_CORES)))
    out = np.zeros((B, S, D), dtype=np.float64)
    for core in range(N_CORES):
        b = core // HG
        out[b] += res.results[core]["outT"].T.astype(np.float64)
    return out.astype(np.float32)
